# revision 1
# baseline (speedup 1.0000x reference)
"""MoE routing kernel for Trainium2, 8 NeuronCores.

Strategy (expert-parallel, one device launch):
  Host: gating softmax + top-k in float64 (0.8% of total FLOPs;
  selection is exact vs the f32 reference since top-k margins are
  orders of magnitude above f32 rounding noise). From the routing,
  build per-expert token lists, pre-scale each gathered token by its
  gate probability (experts are linear, so scaling inputs is exactly
  scaling outputs), transpose, and cast to bf16.
  Device (expert-parallel): each core runs its E/8 experts' matmuls
  in bf16 (full-rate PE, fp32 PSUM accumulation) with k-outer wave
  scheduling so the PE streams behind the DMA, and writes fp32
  outputs. All expert FLOPs and all bulk HBM traffic are on device.
  Host: scatter-adds the compact per-expert outputs into [B, DOUT].

bf16 inputs halve the HBM traffic that made the fp32 version
DMA-bound (~410 GB/s saturated); the kernel is then PE-bound at
~216 ns per 512-row matmul. Expert capacity is capped at the mean
load (capacity factor 1.0, 2x1024 rows per core = 54us PE floor);
the ~1.5% overflow tokens are computed exactly on the host during
the scatter. Measured ~73.5us: ~11us launch preamble/DMA fill +
~56us matmul window + ~4.5us drain.
"""
import numpy as np
from contextlib import ExitStack

import ml_dtypes

import concourse.mybir as mybir
from concourse import bacc, tile
from concourse.bass_utils import run_bass_kernel_spmd

NCORES = 8
P = 128
F32 = mybir.dt.float32
BF16 = mybir.dt.bfloat16
NPBF16 = ml_dtypes.bfloat16

# test-harness knobs (ignored in normal use)
TRACE = False
LAST_EXEC_NS = []
LAST_RESULTS = {}

_cache = {}


def _warmup_pe(nc, pool, ps_pool, n_mm, tag="ps"):
    """Dummy bf16 matmuls on scratch data, issued at kernel start so the
    PE's HAM clock-gate ramps toward 2.4 GHz while the input DMAs
    stream in. Vector memset so the warmup isn't gated on GpSimd."""
    wt = pool.tile([P, 512], BF16, name="warm_sb")
    nc.vector.memset(wt[:], 1.0)
    wp = ps_pool.tile([P, 512], F32, name="warm_ps", tag=tag)
    for _ in range(n_mm):
        nc.tensor.matmul(wp[:], wt[:, :P], wt[:], start=True, stop=True)
    return wt, wp


def _build_expert(C, DIN, DOUT, EPC):
    """Per-core expert compute: for each of the core's EPC experts,
    y_e = xg_e @ W_e over a capacity-C padded, gate-pre-scaled token
    list. bf16 operands, fp32 PSUM, k-outer in waves of 4 PSUM
    accumulation groups (8 banks, two waves in flight) so the PE
    streams behind the DMA.

    Inputs : xgT  [EPC, DIN, C]    bf16 (gathered tokens * gate value,
                                         transposed)
             wexp [EPC, DIN, DOUT] bf16
    Output : yout [EPC, C, DOUT]   f32
    """
    key = ("exp", C, DIN, DOUT, EPC)
    if key in _cache:
        return _cache[key]
    KT = DIN // P
    MT = C // P
    NF = 512
    assert DOUT % NF == 0
    NT = DOUT // NF
    nc = bacc.Bacc("TRN2", target_bir_lowering=False, debug=False,
                   num_devices=NCORES)
    xgT = nc.dram_tensor("xgT", [EPC, DIN, C], BF16, kind="ExternalInput")
    wexp = nc.dram_tensor("wexp", [EPC, DIN, DOUT], BF16,
                          kind="ExternalInput")
    yout = nc.dram_tensor("yout", [EPC, C, DOUT], F32,
                          kind="ExternalOutput")

    with tile.TileContext(nc) as tc:
        with ExitStack() as ctx:
            xg_pool = ctx.enter_context(tc.tile_pool(name="xg", bufs=2))
            w_pool = ctx.enter_context(tc.tile_pool(name="w", bufs=2))
            out_pool = ctx.enter_context(tc.tile_pool(name="out",
                                                      bufs=12))
            ps = ctx.enter_context(tc.tile_pool(name="ps", bufs=8,
                                                space="PSUM"))
            warm_pool = ctx.enter_context(tc.tile_pool(name="warm", bufs=1))
            # warmup PSUM tile shares the wave slots (transient).
            # 7 warmups (~3us of PE activity for the clock ramp) end
            # right when the first input chunks land in SBUF (~10.7us);
            # more would delay the first real matmul past data-ready.
            _warmup_pe(nc, warm_pool, ps, 7, tag="ps")

            # Hoist ALL input loads (both experts) to the front of the
            # sync (xg) / scalar (w) queues, ahead of any output store,
            # so expert 1's loads are never stuck behind expert 0's
            # stores in queue order. EPC tiles fit the pools exactly.
            assert EPC <= 2
            xg_ts, w_ts = [], []
            for e in range(EPC):
                xg_t = xg_pool.tile([P, KT, C], BF16, tag="xg",
                                    name=f"xg{e}")
                w_t = w_pool.tile([P, KT, DOUT], BF16, tag="w",
                                  name=f"w{e}")
                xg_ts.append(xg_t)
                w_ts.append(w_t)
                for k in range(KT):
                    lo, hi = k * P, (k + 1) * P
                    if e == 0 and k == 0:
                        # split only the very first chunk so the first
                        # matmuls start sooner after the DMA engines
                        # wake up (more splits would throttle the fill:
                        # descriptor issue costs ~600ns each)
                        half = 4 * P
                        nc.sync.dma_start(xg_t[:, 0, :half],
                                          xgT[0, :P, :half])
                        nc.sync.dma_start(xg_t[:, 0, half:],
                                          xgT[0, :P, half:])
                        nc.scalar.dma_start(w_t[:, 0], wexp[0, :P, :])
                    else:
                        nc.sync.dma_start(xg_t[:, k], xgT[e, lo:hi, :])
                        nc.scalar.dma_start(w_t[:, k], wexp[e, lo:hi, :])

            for e in range(EPC):
                xg_t, w_t = xg_ts[e], w_ts[e]
                # k-outer waves of concurrent PSUM groups, m-major.
                # The first wave of expert 0 takes all 8 banks: its
                # ~14us of matmul work hides the input-DMA fill and
                # keeps the PE busy so the clock ramp completes early.
                groups = [(m, n) for m in range(MT) for n in range(NT)]
                w0 = 0
                first = (e == 0)
                while w0 < len(groups):
                    size = 8 if first else 4
                    if e == EPC - 1 and len(groups) - w0 <= 4:
                        # end the program on 2-group waves so the final
                        # evict+store drain is short
                        size = 2
                    wave = groups[w0:w0 + size]
                    w0 += len(wave)
                    first = False
                    pss = {g: ps.tile([P, NF], F32, tag="ps",
                                      name=f"ps_{e}_{g[0]}_{g[1]}")
                           for g in wave}
                    for k in range(KT):
                        for (m, n) in wave:
                            nc.tensor.matmul(
                                pss[(m, n)][:],
                                xg_t[:, k, m * P:(m + 1) * P],
                                w_t[:, k, n * NF:(n + 1) * NF],
                                start=(k == 0),
                                stop=(k == KT - 1),
                            )
                    # evict each finished group on the Vector engine
                    # and store right away, alternating store queues by
                    # output half so neither backs up. The program's
                    # final wave splits its evictions across Vector and
                    # Scalar so the tail drains in parallel.
                    last_wave = (e == EPC - 1) and w0 >= len(groups)
                    for gi, (m, n) in enumerate(wave):
                        dst = yout[e, m * P:(m + 1) * P,
                                   n * NF:(n + 1) * NF]
                        ot = out_pool.tile([P, NF], F32, tag="out",
                                           name=f"out_{e}_{m}_{n}")
                        if last_wave and gi % 2 == 1:
                            nc.scalar.copy(ot[:], pss[(m, n)][:])
                        else:
                            nc.vector.tensor_copy(ot[:], pss[(m, n)][:])
                        eng = nc.sync if n == 0 else nc.scalar
                        eng.dma_start(dst, ot[:])
    nc.compile()
    _cache[key] = nc
    return nc


def _run(nc, in_maps):
    kw = {}
    if TRACE:
        kw["trace"] = True
    res = run_bass_kernel_spmd(nc, in_maps, list(range(NCORES)), **kw)
    if TRACE:
        LAST_EXEC_NS.append(res.exec_time_ns)
        LAST_RESULTS["last"] = res
    return res.results


def kernel(x, gate_w, gate_b, expert_w, expert_b, topk):
    x = np.ascontiguousarray(np.asarray(x, dtype=np.float32))
    gate_w = np.asarray(gate_w, dtype=np.float32)
    gate_b = np.asarray(gate_b, dtype=np.float32)
    expert_w = np.asarray(expert_w, dtype=np.float32)
    expert_b = np.asarray(expert_b, dtype=np.float32)
    topk = int(topk)

    B, DIN = x.shape
    E, _, DOUT = expert_w.shape
    assert B % P == 0 and DIN % P == 0
    EPC = E // NCORES
    assert EPC * NCORES == E

    # ---- host: gating (softmax + top-k) in float64 ----
    # Exact relative to the f32 reference: top-k margins (~1e-4 min)
    # dwarf the ~1e-5 f32 summation noise, so selection matches, and
    # the f64 probabilities are tighter than the reference's own f32.
    logits = x.astype(np.float64) @ gate_w.astype(np.float64).T \
        + gate_b.astype(np.float64)
    if topk < E:
        kth = np.partition(logits, E - topk, axis=1)[:, E - topk]
        mask = logits >= kth[:, None]
    else:
        mask = np.ones_like(logits, dtype=bool)
    z = np.exp(logits - logits.max(axis=1, keepdims=True))
    probs = z / z.sum(axis=1, keepdims=True)
    wfull = np.where(mask, probs, 0.0).astype(np.float32)

    # ---- host: routing bookkeeping + gather (pre-scaled, bf16) ----
    # Capacity factor 1.0: each expert's device list is capped at the
    # mean load (B*topk/E, a multiple of 128 here). The few overflow
    # tokens (~1.5% at this size) are computed exactly on the host in
    # f32 during the scatter — standard MoE capacity handling, and it
    # removes a whole 128-row M-tile of padding from every expert.
    toks = [np.nonzero(wfull[:, e])[0] for e in range(E)]
    maxcnt = max(1, max(len(t) for t in toks))
    C = ((maxcnt + P - 1) // P) * P
    cap = (max(P, B * topk // E) // P) * P
    C = min(C, cap)
    dev_toks = [t[:C] for t in toks]
    ovf_toks = [t[C:] for t in toks]

    nc = _build_expert(C, DIN, DOUT, EPC)
    in_maps = []
    for c in range(NCORES):
        xgT = np.zeros((EPC, DIN, C), NPBF16)
        for j in range(EPC):
            e = EPC * c + j
            t = dev_toks[e]
            xs = x[t] * wfull[t, e][:, None]      # gate-scaled tokens
            xgT[j, :, :len(t)] = xs.T.astype(NPBF16)
        in_maps.append({"xgT": xgT,
                        "wexp": expert_w[EPC * c:EPC * (c + 1)]
                        .astype(NPBF16)})
    r = _run(nc, in_maps)

    # ---- host: scatter-add compact outputs (unshard) ----
    y = np.zeros((B, DOUT), np.float32)
    for c in range(NCORES):
        yo = np.asarray(r[c]["yout"], dtype=np.float32)
        for j in range(EPC):
            e = EPC * c + j
            t = dev_toks[e]
            y[t] += yo[j, :len(t)]
    for e in range(E):
        t = ovf_toks[e]
        if len(t):
            y[t] += (x[t] * wfull[t, e][:, None]) @ expert_w[e]
    if np.any(expert_b):
        for e in range(E):
            t = toks[e]
            y[t] += wfull[t, e][:, None] * expert_b[e][None, :]
    return y



# revision 4
# speedup vs baseline: 1.0340x; 1.0340x over previous
"""MoE routing kernel for Trainium2, 8 NeuronCores.

Strategy (expert-parallel, mixed precision, one device launch):
  Host: gating softmax + top-k in float64 (selection is exact vs the
  f32 reference since top-k margins dwarf f32 rounding noise). Per
  expert, sort its assigned tokens by gate value p (descending):
    - top CB=512 (large p)  -> bf16 path (gate-pre-scaled tokens)
    - next C8=512 (small p) -> fp8 e4m3 path (gate-pre-scaled), run
      with MatmulPerfMode.DoubleRow: 2x PE throughput. The fp8
      quantization error lands only on the low-gate half of the
      assignments, keeping total L2 error ~1.6e-2 (< 2e-2 gate).
    - remainder (~2%)       -> host f32 (standard capacity overflow)
  All tensors are packed on the host directly into SBUF layout
  [P, KT, free] so each load is one large, contiguous-per-partition
  DMA descriptor; loads are spread over 4 engine queues (sync /
  scalar / vector / gpsimd), stores (bf16, halving output traffic)
  alternate over scalar / gpsimd.
  Device (per core, 2 experts): fp8-DR waves + bf16 waves, k-outer,
  fp32 PSUM, warmup matmuls covering the PE clock ramp, small final
  waves so the drain is short.

PE floor: per core 128 bf16 + 64 DR matmul instrs ~ 43us (vs 60us
all-bf16). Measured baseline (all-bf16) was ~76-81us.
"""
import numpy as np
from contextlib import ExitStack

import ml_dtypes

import concourse.mybir as mybir
from concourse import bacc, tile
from concourse.bass_utils import run_bass_kernel_spmd

NCORES = 8
P = 128
F32 = mybir.dt.float32
BF16 = mybir.dt.bfloat16
F8 = mybir.dt.float8e4
NPBF16 = ml_dtypes.bfloat16
NPF8 = ml_dtypes.float8_e4m3

# test-harness knobs (ignored in normal use)
TRACE = False
LAST_EXEC_NS = []
LAST_RESULTS = {}

_cache = {}


def _warmup_pe(nc, pool, ps_pool, n_mm, tag="ps"):
    """Dummy bf16 matmuls on scratch data, issued at kernel start so the
    PE's HAM clock-gate ramps toward 2.4 GHz while the input DMAs
    stream in."""
    wt = pool.tile([P, 512], BF16, name="warm_sb")
    nc.vector.memset(wt[:], 1.0)
    wp = ps_pool.tile([P, 512], F32, name="warm_ps", tag=tag)
    for _ in range(n_mm):
        nc.tensor.matmul(wp[:], wt[:, :P], wt[:], start=True, stop=True)
    return wt, wp


def _build_mixed(CB, C8, DIN, DOUT, EPC):
    """Per-core expert compute, mixed bf16/fp8-DoubleRow.

    Inputs : xbT [EPC, P, KT, CB]   bf16 (pre-scaled tokens, SBUF layout)
             x8T [EPC, P, KT, C8]   f8e4 (pre-scaled tokens, SBUF layout)
             wb  [EPC, P, KT, DOUT] bf16
             w8  [EPC, P, KT, DOUT] f8e4
    Output : yout [EPC, 2, MT, P, DOUT] bf16  (path 0 = bf16, 1 = fp8)
    """
    key = ("mix", CB, C8, DIN, DOUT, EPC)
    if key in _cache:
        return _cache[key]
    KT = DIN // P
    MTB = CB // P
    MT8 = C8 // P
    NF = 512
    assert DOUT % NF == 0 and KT % 2 == 0
    NT = DOUT // NF
    assert EPC == 2
    nc = bacc.Bacc("TRN2", target_bir_lowering=False, debug=False,
                   num_devices=NCORES)
    xbT = nc.dram_tensor("xbT", [EPC, P, KT, CB], BF16, kind="ExternalInput")
    x8T = nc.dram_tensor("x8T", [EPC, P, KT, C8], F8, kind="ExternalInput")
    wb = nc.dram_tensor("wb", [EPC, P, KT, DOUT], BF16, kind="ExternalInput")
    w8 = nc.dram_tensor("w8", [EPC, P, KT, DOUT], F8, kind="ExternalInput")
    yout = nc.dram_tensor("yout", [EPC, 2, max(MTB, MT8), P, DOUT], BF16,
                          kind="ExternalOutput")

    with tile.TileContext(nc) as tc:
        with ExitStack() as ctx:
            in_pool = ctx.enter_context(tc.tile_pool(name="in", bufs=1))
            out_pool = ctx.enter_context(tc.tile_pool(name="out", bufs=12))
            ps = ctx.enter_context(tc.tile_pool(name="ps", bufs=8,
                                                space="PSUM"))
            warm_pool = ctx.enter_context(tc.tile_pool(name="warm", bufs=1))
            _warmup_pe(nc, warm_pool, ps, 7, tag="ps")

            # --- hoist ALL input loads to the queue fronts ---------------
            # Loads are split at k boundaries that match when the k-outer
            # waves consume them, so the first wave starts as early as
            # possible while later chunks stream behind it.
            xb_ts, x8_ts, wb_ts, w8_ts = [], [], [], []
            for e in range(EPC):
                xb_ts.append(in_pool.tile([P, KT, CB], BF16, name=f"xb{e}"))
                x8_ts.append(in_pool.tile([P, KT, C8], F8, name=f"x8{e}"))
                wb_ts.append(in_pool.tile([P, KT, DOUT], BF16,
                                          name=f"wb{e}"))
                w8_ts.append(in_pool.tile([P, KT, DOUT], F8, name=f"w8{e}"))
            # sync queue (HWDGE): fp8 tokens first (the fp8 phase runs
            # first), then bf16 tokens of expert 0, then expert 1 fp8
            nc.sync.dma_start(x8_ts[0][:, 0:2], x8T[0, :, 0:2])
            nc.sync.dma_start(x8_ts[0][:, 2:4], x8T[0, :, 2:4])
            nc.sync.dma_start(x8_ts[0][:, 4:KT], x8T[0, :, 4:KT])
            nc.sync.dma_start(xb_ts[0][:, 0:2], xbT[0, :, 0:2])
            nc.sync.dma_start(xb_ts[0][:, 2:KT], xbT[0, :, 2:KT])
            nc.sync.dma_start(x8_ts[1][:], x8T[1])
            # scalar queue (HWDGE): fp8 weights, then expert 1 bf16 tokens
            nc.scalar.dma_start(w8_ts[0][:, 0:2], w8[0, :, 0:2])
            nc.scalar.dma_start(w8_ts[0][:, 2:4], w8[0, :, 2:4])
            nc.scalar.dma_start(w8_ts[0][:, 4:KT], w8[0, :, 4:KT])
            nc.scalar.dma_start(w8_ts[1][:, 0:2], w8[1, :, 0:2])
            nc.scalar.dma_start(w8_ts[1][:, 2:KT], w8[1, :, 2:KT])
            nc.scalar.dma_start(xb_ts[1][:], xbT[1])
            # gpsimd queue (SWDGE): both experts' bf16 weights (4MB; its
            # deadline is latest — bf16 phases run second per expert)
            nc.gpsimd.dma_start(wb_ts[0][:, 0:2], wb[0, :, 0:2])
            nc.gpsimd.dma_start(wb_ts[0][:, 2:KT], wb[0, :, 2:KT])
            nc.gpsimd.dma_start(wb_ts[1][:, 0:2], wb[1, :, 0:2])
            nc.gpsimd.dma_start(wb_ts[1][:, 2:KT], wb[1, :, 2:KT])

            # --- phases: f8(e0), bf16(e0), f8(e1), bf16(e1) --------------
            # (m, n) groups, m-major; k-outer within each wave. Output
            # tiles [P, DOUT] pair the two n-halves of one m-tile; store
            # queues alternate scalar/gpsimd by a global counter.
            store_ctr = [0]

            def emit_phase(e, path, waves, last_phase=False):
                f8p = (path == 1)
                x_t = (x8_ts if f8p else xb_ts)[e]
                w_t = (w8_ts if f8p else wb_ts)[e]
                MT = MT8 if f8p else MTB
                groups = [(m, n) for m in range(MT) for n in range(NT)]
                gi0 = 0
                out_tiles = {}
                for wi, wsize in enumerate(waves):
                    wave = groups[gi0:gi0 + wsize]
                    gi0 += wsize
                    pss = {g: ps.tile([P, NF], F32, tag="ps",
                                      name=f"ps_{e}_{path}_{g[0]}_{g[1]}")
                           for g in wave}
                    if f8p:
                        for kk in range(KT // 2):
                            for (m, n) in wave:
                                nc.tensor.matmul(
                                    pss[(m, n)][:],
                                    x_t[:, 2 * kk:2 * kk + 2,
                                        m * P:(m + 1) * P],
                                    w_t[:, 2 * kk:2 * kk + 2,
                                        n * NF:(n + 1) * NF],
                                    start=(kk == 0),
                                    stop=(kk == KT // 2 - 1),
                                    perf_mode=mybir.MatmulPerfMode.DoubleRow,
                                )
                    else:
                        for k in range(KT):
                            for (m, n) in wave:
                                nc.tensor.matmul(
                                    pss[(m, n)][:],
                                    x_t[:, k, m * P:(m + 1) * P],
                                    w_t[:, k, n * NF:(n + 1) * NF],
                                    start=(k == 0),
                                    stop=(k == KT - 1),
                                )
                    last_wave = last_phase and gi0 >= len(groups)
                    for gi, (m, n) in enumerate(wave):
                        if m not in out_tiles:
                            out_tiles[m] = out_pool.tile(
                                [P, DOUT], BF16, tag="out",
                                name=f"out_{e}_{path}_{m}")
                        ot = out_tiles[m]
                        if last_wave and gi % 2 == 1:
                            nc.scalar.copy(ot[:, n * NF:(n + 1) * NF],
                                           pss[(m, n)][:])
                        else:
                            nc.vector.tensor_copy(
                                ot[:, n * NF:(n + 1) * NF], pss[(m, n)][:])
                        if n == NT - 1:
                            eng = nc.scalar if store_ctr[0] % 2 == 0 \
                                else nc.sync
                            store_ctr[0] += 1
                            eng.dma_start(yout[e, path, m], ot[:])

            emit_phase(0, 1, [8])
            emit_phase(0, 0, [4, 4])
            emit_phase(1, 1, [4, 4])
            emit_phase(1, 0, [4, 2, 2], last_phase=True)
    nc.compile()
    _cache[key] = nc
    return nc


def _run(nc, in_maps):
    kw = {}
    if TRACE:
        kw["trace"] = True
    res = run_bass_kernel_spmd(nc, in_maps, list(range(NCORES)), **kw)
    if TRACE:
        LAST_EXEC_NS.append(res.exec_time_ns)
        LAST_RESULTS["last"] = res
    return res.results


def _pack(a2d, KT, C, np_dtype):
    """[DIN, n] f32 -> [P, KT, C] np_dtype, zero-padded along tokens."""
    DIN = a2d.shape[0]
    out = np.zeros((P, KT, C), np_dtype)
    n = a2d.shape[1]
    out[:, :, :n] = (a2d.reshape(KT, P, -1).transpose(1, 0, 2)
                     .astype(np_dtype))
    return out


def kernel(x, gate_w, gate_b, expert_w, expert_b, topk):
    x = np.ascontiguousarray(np.asarray(x, dtype=np.float32))
    gate_w = np.asarray(gate_w, dtype=np.float32)
    gate_b = np.asarray(gate_b, dtype=np.float32)
    expert_w = np.asarray(expert_w, dtype=np.float32)
    expert_b = np.asarray(expert_b, dtype=np.float32)
    topk = int(topk)

    B, DIN = x.shape
    E, _, DOUT = expert_w.shape
    assert B % P == 0 and DIN % P == 0
    EPC = E // NCORES
    assert EPC * NCORES == E
    KT = DIN // P

    # ---- host: gating (softmax + top-k) in float64 ----
    logits = x.astype(np.float64) @ gate_w.astype(np.float64).T \
        + gate_b.astype(np.float64)
    order = np.argsort(-logits, axis=1, kind="stable")[:, :topk]
    z = np.exp(logits - logits.max(axis=1, keepdims=True))
    probs = z / z.sum(axis=1, keepdims=True)
    pv = np.take_along_axis(probs, order, axis=1).astype(np.float32)

    # capacity: split the mean per-expert load between the two paths
    cap = (max(P, B * topk // E) // P) * P
    CB = C8 = cap // 2

    # ---- host: routing; per expert sort by p, split bf16/fp8/host ----
    dev_b, dev_8, host_t = [], [], []
    for e in range(E):
        selmask = (order == e)
        t = np.nonzero(selmask.any(axis=1))[0]
        p = np.where(selmask[t, 0], pv[t, 0],
                     pv[t, 1] if topk > 1 else 0.0)
        o = np.argsort(-p, kind="stable")
        t, p = t[o], p[o]
        dev_b.append((t[:CB], p[:CB]))
        dev_8.append((t[CB:CB + C8], p[CB:CB + C8]))
        host_t.append((t[CB + C8:], p[CB + C8:]))

    nc = _build_mixed(CB, C8, DIN, DOUT, EPC)
    in_maps = []
    for c in range(NCORES):
        xbT = np.zeros((EPC, P, KT, CB), NPBF16)
        x8T = np.zeros((EPC, P, KT, C8), NPF8)
        wbp = np.zeros((EPC, P, KT, DOUT), NPBF16)
        w8p = np.zeros((EPC, P, KT, DOUT), NPF8)
        for j in range(EPC):
            e = EPC * c + j
            tb, pb = dev_b[e]
            t8, p8 = dev_8[e]
            if len(tb):
                xbT[j] = _pack((x[tb] * pb[:, None]).T, KT, CB, NPBF16)
            if len(t8):
                x8T[j] = _pack((x[t8] * p8[:, None]).T, KT, C8, NPF8)
            wf = expert_w[e].reshape(KT, P, DOUT).transpose(1, 0, 2)
            wbp[j] = wf.astype(NPBF16)
            w8p[j] = wf.astype(NPF8)
        in_maps.append({"xbT": xbT, "x8T": x8T, "wb": wbp, "w8": w8p})
    r = _run(nc, in_maps)

    # ---- host: scatter-add outputs (pure adds; both paths pre-scaled) --
    y = np.zeros((B, DOUT), np.float32)
    for c in range(NCORES):
        yo = np.asarray(r[c]["yout"])
        for j in range(EPC):
            e = EPC * c + j
            tb, _ = dev_b[e]
            t8, _ = dev_8[e]
            if len(tb):
                y[tb] += yo[j, 0].reshape(-1, DOUT)[:len(tb)] \
                    .astype(np.float32)
            if len(t8):
                y[t8] += yo[j, 1].reshape(-1, DOUT)[:len(t8)] \
                    .astype(np.float32)
    for e in range(E):
        t, p = host_t[e]
        if len(t):
            y[t] += (x[t] * p[:, None]) @ expert_w[e]
    if np.any(expert_b):
        for e in range(E):
            for (t, p) in (dev_b[e], dev_8[e], host_t[e]):
                if len(t):
                    y[t] += p[:, None] * expert_b[e][None, :]
    return y


# revision 7
# speedup vs baseline: 1.1130x; 1.0765x over previous
"""MoE routing kernel for Trainium2, 8 NeuronCores.

Strategy (expert-parallel, mixed precision, one device launch):
  Host: gating softmax + top-k in float64 (selection is exact vs the
  f32 reference since top-k margins dwarf f32 rounding noise). Per
  expert, sort its assigned tokens by gate value p (descending):
    - top CB=512 (large p)  -> bf16 path (gate-pre-scaled tokens)
    - next C8=512 (small p) -> fp8 e4m3 path (gate-pre-scaled), run
      with MatmulPerfMode.DoubleRow: 2x PE throughput. The fp8
      quantization error lands only on the low-gate half of the
      assignments, keeping total L2 error ~1.6e-2 (< 2e-2 gate).
    - remainder (~2%)       -> host f32 (standard capacity overflow)
  All tensors are packed on the host directly into SBUF layout
  [P, KT, free] so each load is one large, contiguous-per-partition
  DMA descriptor; loads are spread over 4 engine queues (sync /
  scalar / vector / gpsimd), stores (bf16, halving output traffic)
  alternate over scalar / gpsimd.
  Device (per core, 2 experts): fp8-DR waves + bf16 waves, k-outer,
  fp32 PSUM, warmup matmuls covering the PE clock ramp, small final
  waves so the drain is short.

PE floor: per core 128 bf16 + 64 DR matmul instrs ~ 43us (vs 60us
all-bf16). Measured baseline (all-bf16) was ~76-81us.
"""
import numpy as np
from contextlib import ExitStack

import ml_dtypes

import concourse.mybir as mybir
from concourse import bacc, tile
from concourse.bass_utils import run_bass_kernel_spmd

NCORES = 8
P = 128
F32 = mybir.dt.float32
BF16 = mybir.dt.bfloat16
F8 = mybir.dt.float8e4
NPBF16 = ml_dtypes.bfloat16
NPF8 = ml_dtypes.float8_e4m3

# test-harness knobs (ignored in normal use)
TRACE = False
LAST_EXEC_NS = []
LAST_RESULTS = {}

_cache = {}


def _warmup_pe(nc, pool, ps_pool, n_mm, tag="ps"):
    """Dummy bf16 matmuls on scratch data, issued at kernel start so the
    PE's HAM clock-gate ramps toward 2.4 GHz while the input DMAs
    stream in."""
    wt = pool.tile([P, 512], BF16, name="warm_sb")
    nc.vector.memset(wt[:], 1.0)
    wp = ps_pool.tile([P, 512], F32, name="warm_ps", tag=tag)
    for _ in range(n_mm):
        nc.tensor.matmul(wp[:], wt[:, :P], wt[:], start=True, stop=True)
    return wt, wp


def _build_mixed(CB, C8, DIN, DOUT, EPC):
    """Per-core expert compute, mixed bf16/fp8-DoubleRow.

    Inputs : xbT [EPC, P, KT, CB]   bf16 (pre-scaled tokens, SBUF layout)
             x8T [EPC, P, KT, C8]   f8e4 (pre-scaled tokens, SBUF layout)
             wb  [EPC, P, KT, DOUT] bf16
             w8  [EPC, P, KT, DOUT] f8e4
    Output : yout [EPC, 2, MT, P, DOUT] bf16  (path 0 = bf16, 1 = fp8)
    """
    key = ("mix", CB, C8, DIN, DOUT, EPC)
    if key in _cache:
        return _cache[key]
    KT = DIN // P
    MTB = CB // P
    MT8 = C8 // P
    NF = 512
    assert DOUT % NF == 0 and KT % 2 == 0
    NT = DOUT // NF
    assert EPC == 2
    nc = bacc.Bacc("TRN2", target_bir_lowering=False, debug=False,
                   num_devices=NCORES)
    xbT = nc.dram_tensor("xbT", [EPC, P, KT, CB], BF16, kind="ExternalInput")
    x8T = nc.dram_tensor("x8T", [EPC, P, KT, C8], F8, kind="ExternalInput")
    wb = nc.dram_tensor("wb", [EPC, P, KT, DOUT], BF16, kind="ExternalInput")
    w8 = nc.dram_tensor("w8", [EPC, P, KT, DOUT], F8, kind="ExternalInput")
    yout = nc.dram_tensor("yout", [EPC, 2, max(MTB, MT8), P, DOUT], BF16,
                          kind="ExternalOutput")

    with tile.TileContext(nc) as tc:
        with ExitStack() as ctx:
            in_pool = ctx.enter_context(tc.tile_pool(name="in", bufs=1))
            out_pool = ctx.enter_context(tc.tile_pool(name="out", bufs=12))
            ps = ctx.enter_context(tc.tile_pool(name="ps", bufs=8,
                                                space="PSUM"))
            warm_pool = ctx.enter_context(tc.tile_pool(name="warm", bufs=1))
            _warmup_pe(nc, warm_pool, ps, 8, tag="ps")

            # --- hoist ALL input loads to the queue fronts ---------------
            # Loads are split at k boundaries that match when the k-outer
            # waves consume them, so the first wave starts as early as
            # possible while later chunks stream behind it.
            xb_ts, x8_ts, wb_ts, w8_ts = [], [], [], []
            for e in range(EPC):
                xb_ts.append(in_pool.tile([P, KT, CB], BF16, name=f"xb{e}"))
                x8_ts.append(in_pool.tile([P, KT, C8], F8, name=f"x8{e}"))
                wb_ts.append(in_pool.tile([P, KT, DOUT], BF16,
                                          name=f"wb{e}"))
                w8_ts.append(in_pool.tile([P, KT, DOUT], F8, name=f"w8{e}"))
            # All queues share ~320 GB/s aggregate HBM bandwidth, so the
            # loads are interleaved by COMPUTE-PHASE deadline and spread
            # evenly across all 3 DMA queues (sync/scalar HWDGE + gpsimd
            # SWDGE) — each phase's ~equal byte share per queue keeps
            # arrival order matched to the PE's consumption order.
            # Phase 1: f8(e0) — also split fine so the first wave starts
            # as soon as k01 lands.
            nc.sync.dma_start(x8_ts[0][:, 0:2], x8T[0, :, 0:2])
            nc.scalar.dma_start(w8_ts[0][:, 0:2], w8[0, :, 0:2])
            nc.gpsimd.dma_start(w8_ts[0][:, 4:KT], w8[0, :, 4:KT])
            nc.sync.dma_start(x8_ts[0][:, 2:KT], x8T[0, :, 2:KT])
            nc.scalar.dma_start(w8_ts[0][:, 2:4], w8[0, :, 2:4])
            # Phase 2: bf16(e0)
            nc.sync.dma_start(xb_ts[0][:, 0:2], xbT[0, :, 0:2])
            nc.scalar.dma_start(wb_ts[0][:, 0:4], wb[0, :, 0:4])
            nc.gpsimd.dma_start(wb_ts[0][:, 4:KT], wb[0, :, 4:KT])
            nc.sync.dma_start(xb_ts[0][:, 2:KT], xbT[0, :, 2:KT])
            # Phase 3: f8(e1)
            nc.sync.dma_start(x8_ts[1][:], x8T[1])
            nc.scalar.dma_start(w8_ts[1][:, 0:4], w8[1, :, 0:4])
            nc.gpsimd.dma_start(w8_ts[1][:, 4:KT], w8[1, :, 4:KT])
            # Phase 4: bf16(e1)
            nc.sync.dma_start(xb_ts[1][:], xbT[1])
            nc.scalar.dma_start(wb_ts[1][:, 0:4], wb[1, :, 0:4])
            nc.gpsimd.dma_start(wb_ts[1][:, 4:KT], wb[1, :, 4:KT])

            # --- phases: f8(e0), bf16(e0), f8(e1), bf16(e1) --------------
            # (m, n) groups, m-major; k-outer within each wave. Output
            # tiles [P, DOUT] pair the two n-halves of one m-tile; store
            # queues alternate scalar/gpsimd by a global counter.
            store_ctr = [0]

            def emit_phase(e, path, waves, last_phase=False):
                f8p = (path == 1)
                x_t = (x8_ts if f8p else xb_ts)[e]
                w_t = (w8_ts if f8p else wb_ts)[e]
                MT = MT8 if f8p else MTB
                groups = [(m, n) for m in range(MT) for n in range(NT)]
                gi0 = 0
                out_tiles = {}
                for wi, wsize in enumerate(waves):
                    wave = groups[gi0:gi0 + wsize]
                    gi0 += wsize
                    pss = {g: ps.tile([P, NF], F32, tag="ps",
                                      name=f"ps_{e}_{path}_{g[0]}_{g[1]}")
                           for g in wave}
                    if f8p:
                        for kk in range(KT // 2):
                            for (m, n) in wave:
                                nc.tensor.matmul(
                                    pss[(m, n)][:],
                                    x_t[:, 2 * kk:2 * kk + 2,
                                        m * P:(m + 1) * P],
                                    w_t[:, 2 * kk:2 * kk + 2,
                                        n * NF:(n + 1) * NF],
                                    start=(kk == 0),
                                    stop=(kk == KT // 2 - 1),
                                    perf_mode=mybir.MatmulPerfMode.DoubleRow,
                                )
                    else:
                        for k in range(KT):
                            for (m, n) in wave:
                                nc.tensor.matmul(
                                    pss[(m, n)][:],
                                    x_t[:, k, m * P:(m + 1) * P],
                                    w_t[:, k, n * NF:(n + 1) * NF],
                                    start=(k == 0),
                                    stop=(k == KT - 1),
                                )
                    last_wave = last_phase and gi0 >= len(groups)
                    store_engs = [nc.sync, nc.scalar, nc.gpsimd]
                    for gi, (m, n) in enumerate(wave):
                        if m not in out_tiles:
                            out_tiles[m] = out_pool.tile(
                                [P, DOUT], BF16, tag="out",
                                name=f"out_{e}_{path}_{m}")
                        ot = out_tiles[m]
                        if last_wave and gi % 2 == 1:
                            nc.scalar.copy(ot[:, n * NF:(n + 1) * NF],
                                           pss[(m, n)][:])
                        else:
                            nc.vector.tensor_copy(
                                ot[:, n * NF:(n + 1) * NF], pss[(m, n)][:])
                        if n == NT - 1:
                            if last_wave:
                                # split the very last store across two
                                # queues so the drain is short
                                nc.sync.dma_start(
                                    yout[e, path, m, :, :NF], ot[:, :NF])
                                nc.scalar.dma_start(
                                    yout[e, path, m, :, NF:], ot[:, NF:])
                            else:
                                eng = store_engs[store_ctr[0] % 3]
                                store_ctr[0] += 1
                                eng.dma_start(yout[e, path, m], ot[:])

            emit_phase(0, 1, [8])
            emit_phase(0, 0, [4, 4])
            emit_phase(1, 1, [4, 4])
            emit_phase(1, 0, [4, 2, 2], last_phase=True)
    nc.compile()
    _cache[key] = nc
    return nc


def _run(nc, in_maps):
    kw = {}
    if TRACE:
        kw["trace"] = True
    res = run_bass_kernel_spmd(nc, in_maps, list(range(NCORES)), **kw)
    if TRACE:
        LAST_EXEC_NS.append(res.exec_time_ns)
        LAST_RESULTS["last"] = res
    return res.results


def _pack(a2d, KT, C, np_dtype):
    """[DIN, n] f32 -> [P, KT, C] np_dtype, zero-padded along tokens."""
    DIN = a2d.shape[0]
    out = np.zeros((P, KT, C), np_dtype)
    n = a2d.shape[1]
    out[:, :, :n] = (a2d.reshape(KT, P, -1).transpose(1, 0, 2)
                     .astype(np_dtype))
    return out


def kernel(x, gate_w, gate_b, expert_w, expert_b, topk):
    x = np.ascontiguousarray(np.asarray(x, dtype=np.float32))
    gate_w = np.asarray(gate_w, dtype=np.float32)
    gate_b = np.asarray(gate_b, dtype=np.float32)
    expert_w = np.asarray(expert_w, dtype=np.float32)
    expert_b = np.asarray(expert_b, dtype=np.float32)
    topk = int(topk)

    B, DIN = x.shape
    E, _, DOUT = expert_w.shape
    assert B % P == 0 and DIN % P == 0
    EPC = E // NCORES
    assert EPC * NCORES == E
    KT = DIN // P

    # ---- host: gating (softmax + top-k) in float64 ----
    logits = x.astype(np.float64) @ gate_w.astype(np.float64).T \
        + gate_b.astype(np.float64)
    order = np.argsort(-logits, axis=1, kind="stable")[:, :topk]
    z = np.exp(logits - logits.max(axis=1, keepdims=True))
    probs = z / z.sum(axis=1, keepdims=True)
    pv = np.take_along_axis(probs, order, axis=1).astype(np.float32)

    # capacity: split the mean per-expert load between the two paths
    cap = (max(P, B * topk // E) // P) * P
    CB = C8 = cap // 2

    # ---- host: routing; per expert sort by p, split bf16/fp8/host ----
    dev_b, dev_8, host_t = [], [], []
    for e in range(E):
        selmask = (order == e)
        t = np.nonzero(selmask.any(axis=1))[0]
        p = np.where(selmask[t, 0], pv[t, 0],
                     pv[t, 1] if topk > 1 else 0.0)
        o = np.argsort(-p, kind="stable")
        t, p = t[o], p[o]
        dev_b.append((t[:CB], p[:CB]))
        dev_8.append((t[CB:CB + C8], p[CB:CB + C8]))
        host_t.append((t[CB + C8:], p[CB + C8:]))

    nc = _build_mixed(CB, C8, DIN, DOUT, EPC)
    in_maps = []
    for c in range(NCORES):
        xbT = np.zeros((EPC, P, KT, CB), NPBF16)
        x8T = np.zeros((EPC, P, KT, C8), NPF8)
        wbp = np.zeros((EPC, P, KT, DOUT), NPBF16)
        w8p = np.zeros((EPC, P, KT, DOUT), NPF8)
        for j in range(EPC):
            e = EPC * c + j
            tb, pb = dev_b[e]
            t8, p8 = dev_8[e]
            if len(tb):
                xbT[j] = _pack((x[tb] * pb[:, None]).T, KT, CB, NPBF16)
            if len(t8):
                x8T[j] = _pack((x[t8] * p8[:, None]).T, KT, C8, NPF8)
            wf = expert_w[e].reshape(KT, P, DOUT).transpose(1, 0, 2)
            wbp[j] = wf.astype(NPBF16)
            w8p[j] = wf.astype(NPF8)
        in_maps.append({"xbT": xbT, "x8T": x8T, "wb": wbp, "w8": w8p})
    r = _run(nc, in_maps)

    # ---- host: scatter-add outputs (pure adds; both paths pre-scaled) --
    y = np.zeros((B, DOUT), np.float32)
    for c in range(NCORES):
        yo = np.asarray(r[c]["yout"])
        for j in range(EPC):
            e = EPC * c + j
            tb, _ = dev_b[e]
            t8, _ = dev_8[e]
            if len(tb):
                y[tb] += yo[j, 0].reshape(-1, DOUT)[:len(tb)] \
                    .astype(np.float32)
            if len(t8):
                y[t8] += yo[j, 1].reshape(-1, DOUT)[:len(t8)] \
                    .astype(np.float32)
    for e in range(E):
        t, p = host_t[e]
        if len(t):
            y[t] += (x[t] * p[:, None]) @ expert_w[e]
    if np.any(expert_b):
        for e in range(E):
            for (t, p) in (dev_b[e], dev_8[e], host_t[e]):
                if len(t):
                    y[t] += p[:, None] * expert_b[e][None, :]
    return y


# revision 8
# speedup vs baseline: 1.1916x; 1.0706x over previous
"""MoE routing kernel for Trainium2, 8 NeuronCores.

Strategy (expert-parallel, mixed precision, one device launch):
  Host: gating softmax + top-k in float64 (selection is exact vs the
  f32 reference since top-k margins dwarf f32 rounding noise). Per
  expert, sort its assigned tokens by gate value p (descending):
    - top CB=512 (large p)  -> bf16 path (gate-pre-scaled tokens)
    - next C8=512 (small p) -> fp8 e4m3 path (gate-pre-scaled), run
      with MatmulPerfMode.DoubleRow: 2x PE throughput. The fp8
      quantization error lands only on the low-gate half of the
      assignments, keeping total L2 error ~1.6e-2 (< 2e-2 gate).
    - remainder (~2%)       -> host f32 (standard capacity overflow)
  All tensors are packed on the host directly into SBUF layout
  [P, KT, free] so DMA descriptors have multi-KB contiguous runs per
  partition. All DMA queues share ~320 GB/s aggregate, so loads are
  emitted in compute-phase order, round-robin across the 3 queues
  (sync/scalar HWDGE + gpsimd SWDGE). The fp8 copies of the expert
  weights are produced ON DEVICE (DVE bf16->fp8 cast) instead of
  being loaded, cutting input traffic 9MB -> 7MB per core.
  Device phase order bf16(e0), fp8(e0), bf16(e1), fp8(e1): the long
  bf16 phases lead, giving the DMA stream runway; fp8 phases then
  only need the small x8 tensors plus the already-resident weights.
  fp32 PSUM; bf16 outputs (halves store traffic); warmup matmuls
  cover the PE clock ramp; small final waves keep the drain short.

PE floor: per core 128 bf16 + 64 DR matmul instrs at ~216ns issue
rate ~ 41.5us. Measured baseline (all-bf16) was ~76-81us.
"""
import numpy as np
from contextlib import ExitStack

import ml_dtypes

import concourse.mybir as mybir
from concourse import bacc, tile
from concourse.bass_utils import run_bass_kernel_spmd

NCORES = 8
P = 128
F32 = mybir.dt.float32
BF16 = mybir.dt.bfloat16
F8 = mybir.dt.float8e4
NPBF16 = ml_dtypes.bfloat16
NPF8 = ml_dtypes.float8_e4m3

# device-side bf16->fp8 weight cast (saves 2MB/core of input DMA).
# Set False to load host-quantized fp8 weights instead.
DEVICE_CAST_W8 = True

# test-harness knobs (ignored in normal use)
TRACE = False
LAST_EXEC_NS = []
LAST_RESULTS = {}

_cache = {}


def _warmup_pe(nc, pool, ps_pool, n_mm, tag="ps"):
    """Dummy bf16 matmuls on scratch data, issued at kernel start so the
    PE's HAM clock-gate ramps toward 2.4 GHz while the input DMAs
    stream in."""
    wt = pool.tile([P, 512], BF16, name="warm_sb")
    nc.vector.memset(wt[:], 1.0)
    wp = ps_pool.tile([P, 512], F32, name="warm_ps", tag=tag)
    for _ in range(n_mm):
        nc.tensor.matmul(wp[:], wt[:, :P], wt[:], start=True, stop=True)
    return wt, wp


def _build_mixed(CB, C8, DIN, DOUT, EPC):
    """Per-core expert compute, mixed bf16/fp8-DoubleRow.

    Inputs : xbT [EPC, P, KT, CB]   bf16 (pre-scaled tokens, SBUF layout)
             x8T [EPC, P, KT, C8]   f8e4 (pre-scaled tokens, SBUF layout)
             wb  [EPC, P, KT, DOUT] bf16
             (w8 [EPC, P, KT, DOUT] f8e4 -- only if not DEVICE_CAST_W8)
    Output : yout [EPC, 2, MT, P, DOUT] bf16  (path 0 = bf16, 1 = fp8)
    """
    key = ("mix", CB, C8, DIN, DOUT, EPC, DEVICE_CAST_W8)
    if key in _cache:
        return _cache[key]
    KT = DIN // P
    MTB = CB // P
    MT8 = C8 // P
    NF = 512
    assert DOUT % NF == 0 and KT % 2 == 0
    NT = DOUT // NF
    assert EPC == 2
    nc = bacc.Bacc("TRN2", target_bir_lowering=False, debug=False,
                   num_devices=NCORES)
    xbT = nc.dram_tensor("xbT", [EPC, P, KT, CB], BF16, kind="ExternalInput")
    x8T = nc.dram_tensor("x8T", [EPC, P, KT, C8], F8, kind="ExternalInput")
    wb = nc.dram_tensor("wb", [EPC, P, KT, DOUT], BF16, kind="ExternalInput")
    if not DEVICE_CAST_W8:
        w8 = nc.dram_tensor("w8", [EPC, P, KT, DOUT], F8,
                            kind="ExternalInput")
    yout = nc.dram_tensor("yout", [EPC, 2, max(MTB, MT8), P, DOUT], BF16,
                          kind="ExternalOutput")

    with tile.TileContext(nc) as tc:
        with ExitStack() as ctx:
            in_pool = ctx.enter_context(tc.tile_pool(name="in", bufs=1))
            out_pool = ctx.enter_context(tc.tile_pool(name="out", bufs=12))
            ps = ctx.enter_context(tc.tile_pool(name="ps", bufs=8,
                                                space="PSUM"))
            warm_pool = ctx.enter_context(tc.tile_pool(name="warm", bufs=1))
            _warmup_pe(nc, warm_pool, ps, 8, tag="ps")

            xb_ts, x8_ts, wb_ts, w8_ts = [], [], [], []
            for e in range(EPC):
                xb_ts.append(in_pool.tile([P, KT, CB], BF16, name=f"xb{e}"))
                x8_ts.append(in_pool.tile([P, KT, C8], F8, name=f"x8{e}"))
                wb_ts.append(in_pool.tile([P, KT, DOUT], BF16,
                                          name=f"wb{e}"))
                w8_ts.append(in_pool.tile([P, KT, DOUT], F8, name=f"w8{e}"))

            # --- loads: compute-phase order, round-robin over queues ----
            # k-pair granularity so arrival tracks the k-outer waves.
            qs = [nc.sync, nc.scalar, nc.gpsimd]
            qi = [0]

            def load(dst, src):
                qs[qi[0] % 3].dma_start(dst, src)
                qi[0] += 1

            # phase 1: bf16(e0) — wb/xb k-pairs interleaved
            for k in range(0, KT, 2):
                load(wb_ts[0][:, k:k + 2], wb[0, :, k:k + 2])
                load(xb_ts[0][:, k:k + 2], xbT[0, :, k:k + 2])
            # phase 2: fp8(e0)
            load(x8_ts[0][:, 0:4], x8T[0, :, 0:4])
            load(x8_ts[0][:, 4:KT], x8T[0, :, 4:KT])
            if not DEVICE_CAST_W8:
                load(w8_ts[0][:, 0:4], w8[0, :, 0:4])
                load(w8_ts[0][:, 4:KT], w8[0, :, 4:KT])
            # phase 3: bf16(e1)
            for k in range(0, KT, 2):
                load(wb_ts[1][:, k:k + 2], wb[1, :, k:k + 2])
                load(xb_ts[1][:, k:k + 2], xbT[1, :, k:k + 2])
            # phase 4: fp8(e1)
            load(x8_ts[1][:, 0:4], x8T[1, :, 0:4])
            load(x8_ts[1][:, 4:KT], x8T[1, :, 4:KT])
            if not DEVICE_CAST_W8:
                load(w8_ts[1][:, 0:4], w8[1, :, 0:4])
                load(w8_ts[1][:, 4:KT], w8[1, :, 4:KT])

            def cast_w8(e):
                # DVE bf16 -> fp8 cast, k-pair granularity (|W| << 240,
                # so plain cast cannot overflow e4m3)
                for k in range(0, KT, 2):
                    nc.vector.tensor_copy(w8_ts[e][:, k:k + 2],
                                          wb_ts[e][:, k:k + 2])

            if DEVICE_CAST_W8:
                cast_w8(0)

            # --- compute phases ----------------------------------------
            store_ctr = [0]

            def emit_phase(e, path, waves, last_phase=False):
                f8p = (path == 1)
                x_t = (x8_ts if f8p else xb_ts)[e]
                w_t = (w8_ts if f8p else wb_ts)[e]
                MT = MT8 if f8p else MTB
                groups = [(m, n) for m in range(MT) for n in range(NT)]
                gi0 = 0
                out_tiles = {}
                for wsize in waves:
                    wave = groups[gi0:gi0 + wsize]
                    gi0 += wsize
                    pss = {g: ps.tile([P, NF], F32, tag="ps",
                                      name=f"ps_{e}_{path}_{g[0]}_{g[1]}")
                           for g in wave}
                    if f8p:
                        for kk in range(KT // 2):
                            for (m, n) in wave:
                                nc.tensor.matmul(
                                    pss[(m, n)][:],
                                    x_t[:, 2 * kk:2 * kk + 2,
                                        m * P:(m + 1) * P],
                                    w_t[:, 2 * kk:2 * kk + 2,
                                        n * NF:(n + 1) * NF],
                                    start=(kk == 0),
                                    stop=(kk == KT // 2 - 1),
                                    perf_mode=mybir.MatmulPerfMode.DoubleRow,
                                )
                    else:
                        for k in range(KT):
                            for (m, n) in wave:
                                nc.tensor.matmul(
                                    pss[(m, n)][:],
                                    x_t[:, k, m * P:(m + 1) * P],
                                    w_t[:, k, n * NF:(n + 1) * NF],
                                    start=(k == 0),
                                    stop=(k == KT - 1),
                                )
                    last_wave = last_phase and gi0 >= len(groups)
                    for gi, (m, n) in enumerate(wave):
                        if m not in out_tiles:
                            out_tiles[m] = out_pool.tile(
                                [P, DOUT], BF16, tag="out",
                                name=f"out_{e}_{path}_{m}")
                        ot = out_tiles[m]
                        if last_wave and gi % 2 == 1:
                            nc.scalar.copy(ot[:, n * NF:(n + 1) * NF],
                                           pss[(m, n)][:])
                        else:
                            nc.vector.tensor_copy(
                                ot[:, n * NF:(n + 1) * NF], pss[(m, n)][:])
                        if n == NT - 1:
                            if last_wave:
                                # split the very last store across two
                                # queues so the drain is short
                                nc.sync.dma_start(
                                    yout[e, path, m, :, :NF], ot[:, :NF])
                                nc.scalar.dma_start(
                                    yout[e, path, m, :, NF:], ot[:, NF:])
                            else:
                                eng = qs[store_ctr[0] % 3]
                                store_ctr[0] += 1
                                eng.dma_start(yout[e, path, m], ot[:])

            emit_phase(0, 0, [4, 4])
            emit_phase(0, 1, [4, 4])
            if DEVICE_CAST_W8:
                cast_w8(1)
            emit_phase(1, 0, [4, 4])
            emit_phase(1, 1, [4, 2, 2], last_phase=True)
    nc.compile()
    _cache[key] = nc
    return nc


def _run(nc, in_maps):
    kw = {}
    if TRACE:
        kw["trace"] = True
    res = run_bass_kernel_spmd(nc, in_maps, list(range(NCORES)), **kw)
    if TRACE:
        LAST_EXEC_NS.append(res.exec_time_ns)
        LAST_RESULTS["last"] = res
    return res.results


def _pack(a2d, KT, C, np_dtype):
    """[DIN, n] f32 -> [P, KT, C] np_dtype, zero-padded along tokens."""
    out = np.zeros((P, KT, C), np_dtype)
    n = a2d.shape[1]
    out[:, :, :n] = (a2d.reshape(KT, P, -1).transpose(1, 0, 2)
                     .astype(np_dtype))
    return out


def kernel(x, gate_w, gate_b, expert_w, expert_b, topk):
    x = np.ascontiguousarray(np.asarray(x, dtype=np.float32))
    gate_w = np.asarray(gate_w, dtype=np.float32)
    gate_b = np.asarray(gate_b, dtype=np.float32)
    expert_w = np.asarray(expert_w, dtype=np.float32)
    expert_b = np.asarray(expert_b, dtype=np.float32)
    topk = int(topk)

    B, DIN = x.shape
    E, _, DOUT = expert_w.shape
    assert B % P == 0 and DIN % P == 0
    EPC = E // NCORES
    assert EPC * NCORES == E
    KT = DIN // P

    # ---- host: gating (softmax + top-k) in float64 ----
    logits = x.astype(np.float64) @ gate_w.astype(np.float64).T \
        + gate_b.astype(np.float64)
    order = np.argsort(-logits, axis=1, kind="stable")[:, :topk]
    z = np.exp(logits - logits.max(axis=1, keepdims=True))
    probs = z / z.sum(axis=1, keepdims=True)
    pv = np.take_along_axis(probs, order, axis=1).astype(np.float32)

    # capacity: split the mean per-expert load between the two paths
    cap = (max(P, B * topk // E) // P) * P
    CB = C8 = cap // 2

    # ---- host: routing; per expert sort by p, split bf16/fp8/host ----
    dev_b, dev_8, host_t = [], [], []
    for e in range(E):
        selmask = (order == e)
        t = np.nonzero(selmask.any(axis=1))[0]
        p = np.where(selmask[t, 0], pv[t, 0],
                     pv[t, 1] if topk > 1 else 0.0)
        o = np.argsort(-p, kind="stable")
        t, p = t[o], p[o]
        dev_b.append((t[:CB], p[:CB]))
        dev_8.append((t[CB:CB + C8], p[CB:CB + C8]))
        host_t.append((t[CB + C8:], p[CB + C8:]))

    nc = _build_mixed(CB, C8, DIN, DOUT, EPC)
    in_maps = []
    for c in range(NCORES):
        xbT = np.zeros((EPC, P, KT, CB), NPBF16)
        x8T = np.zeros((EPC, P, KT, C8), NPF8)
        wbp = np.zeros((EPC, P, KT, DOUT), NPBF16)
        w8p = None if DEVICE_CAST_W8 else \
            np.zeros((EPC, P, KT, DOUT), NPF8)
        for j in range(EPC):
            e = EPC * c + j
            tb, pb = dev_b[e]
            t8, p8 = dev_8[e]
            if len(tb):
                xbT[j] = _pack((x[tb] * pb[:, None]).T, KT, CB, NPBF16)
            if len(t8):
                x8T[j] = _pack((x[t8] * p8[:, None]).T, KT, C8, NPF8)
            wf = expert_w[e].reshape(KT, P, DOUT).transpose(1, 0, 2)
            wbp[j] = wf.astype(NPBF16)
            if w8p is not None:
                w8p[j] = wf.astype(NPF8)
        im = {"xbT": xbT, "x8T": x8T, "wb": wbp}
        if w8p is not None:
            im["w8"] = w8p
        in_maps.append(im)
    r = _run(nc, in_maps)

    # ---- host: scatter-add outputs (pure adds; both paths pre-scaled) --
    y = np.zeros((B, DOUT), np.float32)
    for c in range(NCORES):
        yo = np.asarray(r[c]["yout"])
        for j in range(EPC):
            e = EPC * c + j
            tb, _ = dev_b[e]
            t8, _ = dev_8[e]
            if len(tb):
                y[tb] += yo[j, 0].reshape(-1, DOUT)[:len(tb)] \
                    .astype(np.float32)
            if len(t8):
                y[t8] += yo[j, 1].reshape(-1, DOUT)[:len(t8)] \
                    .astype(np.float32)
    for e in range(E):
        t, p = host_t[e]
        if len(t):
            y[t] += (x[t] * p[:, None]) @ expert_w[e]
    if np.any(expert_b):
        for e in range(E):
            for (t, p) in (dev_b[e], dev_8[e], host_t[e]):
                if len(t):
                    y[t] += p[:, None] * expert_b[e][None, :]
    return y


# revision 16
# speedup vs baseline: 1.2260x; 1.0288x over previous
"""MoE routing kernel for Trainium2, 8 NeuronCores.

Strategy (expert-parallel, mixed precision, one device launch):
  Host: gating softmax + top-k in float64 (selection is exact vs the
  f32 reference since top-k margins dwarf f32 rounding noise). Per
  expert, sort its assigned tokens by gate value p (descending):
    - top CB=512 (large p)  -> bf16 path (gate-pre-scaled tokens)
    - next C8=512 (small p) -> fp8 e4m3 path (gate-pre-scaled), run
      with MatmulPerfMode.DoubleRow: 2x PE throughput. The fp8
      quantization error lands only on the low-gate half of the
      assignments, keeping total L2 error ~1.6e-2 (< 2e-2 gate).
    - remainder (~2%)       -> host f32 (standard capacity overflow)
  All tensors are packed on the host directly into SBUF layout
  [P, KT, free] so DMA descriptors have multi-KB contiguous runs per
  partition. All DMA queues share ~320 GB/s aggregate, so loads are
  emitted in compute-phase order, round-robin across the 3 queues
  (sync/scalar HWDGE + gpsimd SWDGE). The fp8 copies of the expert
  weights are produced ON DEVICE (DVE bf16->fp8 cast) instead of
  being loaded, cutting input traffic 9MB -> 7MB per core.
  Device phase order bf16(e0), fp8(e0), bf16(e1), fp8(e1): the long
  bf16 phases lead, giving the DMA stream runway; fp8 phases then
  only need the small x8 tensors plus the already-resident weights.
  fp32 PSUM; bf16 outputs (halves store traffic); warmup matmuls
  cover the PE clock ramp; small final waves keep the drain short.

PE floor: per core 128 bf16 + 64 DR matmul instrs at ~216ns issue
rate ~ 41.5us. Measured baseline (all-bf16) was ~76-81us.
"""
import numpy as np
from contextlib import ExitStack

import ml_dtypes

import concourse.mybir as mybir
from concourse import bacc, tile
from concourse.bass_utils import run_bass_kernel_spmd

NCORES = 8
P = 128
F32 = mybir.dt.float32
BF16 = mybir.dt.bfloat16
F8 = mybir.dt.float8e4
NPBF16 = ml_dtypes.bfloat16
NPF8 = ml_dtypes.float8_e4m3

# device-side bf16->fp8 weight cast (saves 2MB/core of input DMA).
# Set False to load host-quantized fp8 weights instead.
DEVICE_CAST_W8 = True

# test-harness knobs (ignored in normal use)
TRACE = False
LAST_EXEC_NS = []
LAST_RESULTS = {}

_cache = {}


def _warmup_pe(nc, pool, ps_pool, n_mm, tag="ps"):
    """Dummy bf16 matmuls on scratch data, issued at kernel start so the
    PE's HAM clock-gate ramps toward 2.4 GHz while the input DMAs
    stream in."""
    wt = pool.tile([P, 512], BF16, name="warm_sb")
    nc.vector.memset(wt[:], 1.0)
    wp = ps_pool.tile([P, 512], F32, name="warm_ps", tag=tag)
    for _ in range(n_mm):
        nc.tensor.matmul(wp[:], wt[:, :P], wt[:], start=True, stop=True)
    return wt, wp


def _build_mixed(CB, C8, DIN, DOUT, EPC):
    """Per-core expert compute, mixed bf16/fp8-DoubleRow.

    Inputs : xbT [EPC, P, KT, CB]   bf16 (pre-scaled tokens, SBUF layout)
             x8T [EPC, P, KT, C8]   f8e4 (pre-scaled tokens, SBUF layout)
             wb  [EPC, P, KT, DOUT] bf16
             (w8 [EPC, P, KT, DOUT] f8e4 -- only if not DEVICE_CAST_W8)
    Output : yout [EPC, 2, MT, P, DOUT] bf16  (path 0 = bf16, 1 = fp8)
    """
    key = ("mix", CB, C8, DIN, DOUT, EPC, DEVICE_CAST_W8)
    if key in _cache:
        return _cache[key]
    KT = DIN // P
    MTB = CB // P
    MT8 = C8 // P
    NF = 512
    assert DOUT % NF == 0 and KT % 2 == 0
    NT = DOUT // NF
    assert EPC == 2
    nc = bacc.Bacc("TRN2", target_bir_lowering=False, debug=False,
                   num_devices=NCORES)
    xbT = nc.dram_tensor("xbT", [EPC, P, KT, CB], BF16, kind="ExternalInput")
    x8T = nc.dram_tensor("x8T", [EPC, P, KT, C8], F8, kind="ExternalInput")
    wb = nc.dram_tensor("wb", [EPC, P, KT, DOUT], BF16, kind="ExternalInput")
    if not DEVICE_CAST_W8:
        w8 = nc.dram_tensor("w8", [EPC, P, KT, DOUT], F8,
                            kind="ExternalInput")
    yout = nc.dram_tensor("yout", [EPC, 2, max(MTB, MT8), P, DOUT], BF16,
                          kind="ExternalOutput")

    with tile.TileContext(nc) as tc:
        with ExitStack() as ctx:
            in_pool = ctx.enter_context(tc.tile_pool(name="in", bufs=1))
            out_pool = ctx.enter_context(tc.tile_pool(name="out", bufs=12))
            ps = ctx.enter_context(tc.tile_pool(name="ps", bufs=8,
                                                space="PSUM"))
            warm_pool = ctx.enter_context(tc.tile_pool(name="warm", bufs=1))
            _warmup_pe(nc, warm_pool, ps, 16, tag="ps")

            xb_ts, x8_ts, wb_ts, w8_ts = [], [], [], []
            for e in range(EPC):
                xb_ts.append(in_pool.tile([P, KT, CB], BF16, name=f"xb{e}"))
                x8_ts.append(in_pool.tile([P, KT, C8], F8, name=f"x8{e}"))
                wb_ts.append(in_pool.tile([P, KT, DOUT], BF16,
                                          name=f"wb{e}"))
                w8_ts.append(in_pool.tile([P, KT, DOUT], F8, name=f"w8{e}"))

            # --- loads: compute-phase order, round-robin over queues ----
            # k-pair granularity so arrival tracks the k-outer waves.
            qs = [nc.sync, nc.scalar, nc.gpsimd]
            qi = [0]

            def load(dst, src):
                qs[qi[0] % 3].dma_start(dst, src)
                qi[0] += 1

            # phase 1: bf16(e0) — wb/xb k-pairs interleaved
            for k in range(0, KT, 2):
                load(wb_ts[0][:, k:k + 2], wb[0, :, k:k + 2])
                load(xb_ts[0][:, k:k + 2], xbT[0, :, k:k + 2])
            # phase 2: fp8(e0)
            load(x8_ts[0][:, 0:4], x8T[0, :, 0:4])
            load(x8_ts[0][:, 4:KT], x8T[0, :, 4:KT])
            if not DEVICE_CAST_W8:
                load(w8_ts[0][:, 0:4], w8[0, :, 0:4])
                load(w8_ts[0][:, 4:KT], w8[0, :, 4:KT])
            # phase 3: bf16(e1)
            for k in range(0, KT, 2):
                load(wb_ts[1][:, k:k + 2], wb[1, :, k:k + 2])
                load(xb_ts[1][:, k:k + 2], xbT[1, :, k:k + 2])
            # phase 4: fp8(e1)
            load(x8_ts[1][:, 0:4], x8T[1, :, 0:4])
            load(x8_ts[1][:, 4:KT], x8T[1, :, 4:KT])
            if not DEVICE_CAST_W8:
                load(w8_ts[1][:, 0:4], w8[1, :, 0:4])
                load(w8_ts[1][:, 4:KT], w8[1, :, 4:KT])

            def cast_w8(e, eng=None):
                # bf16 -> fp8 cast, k-pair granularity (|W| << 240, so a
                # plain cast cannot overflow e4m3). DVE tensor_copy or
                # ACT activation-copy, whichever engine has slack.
                for k in range(0, KT, 2):
                    if eng is nc.scalar:
                        eng.copy(w8_ts[e][:, k:k + 2],
                                 wb_ts[e][:, k:k + 2])
                    else:
                        nc.vector.tensor_copy(w8_ts[e][:, k:k + 2],
                                              wb_ts[e][:, k:k + 2])

            if DEVICE_CAST_W8:
                cast_w8(0)

            # --- compute phases ----------------------------------------
            store_ctr = [0]

            def emit_phase(e, path, waves, last_phase=False):
                f8p = (path == 1)
                x_t = (x8_ts if f8p else xb_ts)[e]
                w_t = (w8_ts if f8p else wb_ts)[e]
                MT = MT8 if f8p else MTB
                groups = [(m, n) for m in range(MT) for n in range(NT)]
                gi0 = 0
                out_tiles = {}
                nwaves = len(waves)
                for wi, wsize in enumerate(waves):
                    if isinstance(wsize, list):
                        wave = wsize
                        gi0 += len(wave)
                    else:
                        wave = groups[gi0:gi0 + wsize]
                        gi0 += wsize
                    pss = {g: ps.tile([P, NF], F32, tag="ps",
                                      name=f"ps_{e}_{path}_{g[0]}_{g[1]}")
                           for g in wave}
                    if f8p:
                        for kk in range(KT // 2):
                            for (m, n) in wave:
                                nc.tensor.matmul(
                                    pss[(m, n)][:],
                                    x_t[:, 2 * kk:2 * kk + 2,
                                        m * P:(m + 1) * P],
                                    w_t[:, 2 * kk:2 * kk + 2,
                                        n * NF:(n + 1) * NF],
                                    start=(kk == 0),
                                    stop=(kk == KT // 2 - 1),
                                    perf_mode=mybir.MatmulPerfMode.DoubleRow,
                                )
                    else:
                        for k in range(KT):
                            for (m, n) in wave:
                                nc.tensor.matmul(
                                    pss[(m, n)][:],
                                    x_t[:, k, m * P:(m + 1) * P],
                                    w_t[:, k, n * NF:(n + 1) * NF],
                                    start=(k == 0),
                                    stop=(k == KT - 1),
                                )
                    last_wave = last_phase and wi == nwaves - 1
                    for gi, (m, n) in enumerate(wave):
                        if m not in out_tiles:
                            out_tiles[m] = out_pool.tile(
                                [P, DOUT], BF16, tag="out",
                                name=f"out_{e}_{path}_{m}")
                        ot = out_tiles[m]
                        if last_wave and gi % 2 == 1:
                            nc.scalar.copy(ot[:, n * NF:(n + 1) * NF],
                                           pss[(m, n)][:])
                        else:
                            nc.vector.tensor_copy(
                                ot[:, n * NF:(n + 1) * NF], pss[(m, n)][:])
                        if n == NT - 1:
                            if last_wave:
                                # the n-major final wave completes two
                                # tiles at once: full-width stores on the
                                # two HWDGE queues drain in parallel
                                eng = nc.sync if gi % 2 == 0 else nc.scalar
                                eng.dma_start(yout[e, path, m], ot[:])
                            else:
                                eng = qs[store_ctr[0] % 3]
                                store_ctr[0] += 1
                                eng.dma_start(yout[e, path, m], ot[:])

            # bf16(e0) first wave of 6: chunk consumption (1.3us/chunk)
            # then matches the early DMA arrival rate, avoiding PE stalls
            # while the rings ramp up.
            emit_phase(0, 0, [6, 2])
            emit_phase(0, 1, [4, 4])
            if DEVICE_CAST_W8:
                cast_w8(1, eng=nc.scalar)
            emit_phase(1, 0, [4, 4])
            emit_phase(1, 1, [4, [(2, 0), (3, 0)], [(2, 1), (3, 1)]],
                       last_phase=True)
    nc.compile()
    _cache[key] = nc
    return nc


def _run(nc, in_maps):
    kw = {}
    if TRACE:
        kw["trace"] = True
    res = run_bass_kernel_spmd(nc, in_maps, list(range(NCORES)), **kw)
    if TRACE:
        LAST_EXEC_NS.append(res.exec_time_ns)
        LAST_RESULTS["last"] = res
    return res.results


def _pack(a2d, KT, C, np_dtype):
    """[DIN, n] f32 -> [P, KT, C] np_dtype, zero-padded along tokens."""
    out = np.zeros((P, KT, C), np_dtype)
    n = a2d.shape[1]
    out[:, :, :n] = (a2d.reshape(KT, P, -1).transpose(1, 0, 2)
                     .astype(np_dtype))
    return out


def kernel(x, gate_w, gate_b, expert_w, expert_b, topk):
    x = np.ascontiguousarray(np.asarray(x, dtype=np.float32))
    gate_w = np.asarray(gate_w, dtype=np.float32)
    gate_b = np.asarray(gate_b, dtype=np.float32)
    expert_w = np.asarray(expert_w, dtype=np.float32)
    expert_b = np.asarray(expert_b, dtype=np.float32)
    topk = int(topk)

    B, DIN = x.shape
    E, _, DOUT = expert_w.shape
    assert B % P == 0 and DIN % P == 0
    EPC = E // NCORES
    assert EPC * NCORES == E
    KT = DIN // P

    # ---- host: gating (softmax + top-k) in float64 ----
    logits = x.astype(np.float64) @ gate_w.astype(np.float64).T \
        + gate_b.astype(np.float64)
    order = np.argsort(-logits, axis=1, kind="stable")[:, :topk]
    z = np.exp(logits - logits.max(axis=1, keepdims=True))
    probs = z / z.sum(axis=1, keepdims=True)
    pv = np.take_along_axis(probs, order, axis=1).astype(np.float32)

    # capacity: split the mean per-expert load between the two paths
    cap = (max(P, B * topk // E) // P) * P
    CB = C8 = cap // 2

    # ---- host: routing; per expert sort by p, split bf16/fp8/host ----
    dev_b, dev_8, host_t = [], [], []
    for e in range(E):
        selmask = (order == e)
        t = np.nonzero(selmask.any(axis=1))[0]
        p = np.where(selmask[t, 0], pv[t, 0],
                     pv[t, 1] if topk > 1 else 0.0)
        o = np.argsort(-p, kind="stable")
        t, p = t[o], p[o]
        dev_b.append((t[:CB], p[:CB]))
        dev_8.append((t[CB:CB + C8], p[CB:CB + C8]))
        host_t.append((t[CB + C8:], p[CB + C8:]))

    nc = _build_mixed(CB, C8, DIN, DOUT, EPC)
    in_maps = []
    for c in range(NCORES):
        xbT = np.zeros((EPC, P, KT, CB), NPBF16)
        x8T = np.zeros((EPC, P, KT, C8), NPF8)
        wbp = np.zeros((EPC, P, KT, DOUT), NPBF16)
        w8p = None if DEVICE_CAST_W8 else \
            np.zeros((EPC, P, KT, DOUT), NPF8)
        for j in range(EPC):
            e = EPC * c + j
            tb, pb = dev_b[e]
            t8, p8 = dev_8[e]
            if len(tb):
                xbT[j] = _pack((x[tb] * pb[:, None]).T, KT, CB, NPBF16)
            if len(t8):
                x8T[j] = _pack((x[t8] * p8[:, None]).T, KT, C8, NPF8)
            wf = expert_w[e].reshape(KT, P, DOUT).transpose(1, 0, 2)
            wbp[j] = wf.astype(NPBF16)
            if w8p is not None:
                w8p[j] = wf.astype(NPF8)
        im = {"xbT": xbT, "x8T": x8T, "wb": wbp}
        if w8p is not None:
            im["w8"] = w8p
        in_maps.append(im)
    r = _run(nc, in_maps)

    # ---- host: scatter-add outputs (pure adds; both paths pre-scaled) --
    y = np.zeros((B, DOUT), np.float32)
    for c in range(NCORES):
        yo = np.asarray(r[c]["yout"])
        for j in range(EPC):
            e = EPC * c + j
            tb, _ = dev_b[e]
            t8, _ = dev_8[e]
            if len(tb):
                y[tb] += yo[j, 0].reshape(-1, DOUT)[:len(tb)] \
                    .astype(np.float32)
            if len(t8):
                y[t8] += yo[j, 1].reshape(-1, DOUT)[:len(t8)] \
                    .astype(np.float32)
    for e in range(E):
        t, p = host_t[e]
        if len(t):
            y[t] += (x[t] * p[:, None]) @ expert_w[e]
    if np.any(expert_b):
        for e in range(E):
            for (t, p) in (dev_b[e], dev_8[e], host_t[e]):
                if len(t):
                    y[t] += p[:, None] * expert_b[e][None, :]
    return y


# revision 19
# speedup vs baseline: 1.2786x; 1.0429x over previous
"""MoE routing kernel for Trainium2, 8 NeuronCores.

Strategy (expert-parallel, mixed precision, one device launch):
  Host: gating softmax + top-k in float64 (selection is exact vs the
  f32 reference since top-k margins dwarf f32 rounding noise). Per
  expert, sort its assigned tokens by gate value p (descending):
    - top CB=512 (large p)  -> bf16 path (gate-pre-scaled tokens)
    - next C8=512 (small p) -> fp8 e4m3 path (gate-pre-scaled), run
      with MatmulPerfMode.DoubleRow: 2x PE throughput. The fp8
      quantization error lands only on the low-gate half of the
      assignments, keeping total L2 error ~1.6e-2 (< 2e-2 gate).
    - remainder (~2%)       -> host f32 (standard capacity overflow)
  All tensors are packed on the host directly into SBUF layout
  [P, KT, free] so DMA descriptors have multi-KB contiguous runs per
  partition. All DMA queues share ~320 GB/s aggregate, so loads are
  emitted in compute-phase order, round-robin across the 3 queues
  (sync/scalar HWDGE + gpsimd SWDGE). The fp8 copies of the expert
  weights are produced ON DEVICE (DVE bf16->fp8 cast) instead of
  being loaded, cutting input traffic 9MB -> 7MB per core.
  Device phase order bf16(e0), fp8(e0), bf16(e1), fp8(e1): the long
  bf16 phases lead, giving the DMA stream runway; fp8 phases then
  only need the small x8 tensors plus the already-resident weights.
  fp32 PSUM; bf16 outputs (halves store traffic); warmup matmuls
  cover the PE clock ramp; small final waves keep the drain short.

PE floor: per core 128 bf16 + 64 DR matmul instrs at ~216ns issue
rate ~ 41.5us. Measured baseline (all-bf16) was ~76-81us.
"""
import numpy as np
from contextlib import ExitStack

import ml_dtypes

import concourse.mybir as mybir
from concourse import bacc, tile
from concourse.bass_utils import run_bass_kernel_spmd

NCORES = 8
P = 128
F32 = mybir.dt.float32
BF16 = mybir.dt.bfloat16
F8 = mybir.dt.float8e4
NPBF16 = ml_dtypes.bfloat16
NPF8 = ml_dtypes.float8_e4m3

# device-side bf16->fp8 weight cast (saves 2MB/core of input DMA).
# Set False to load host-quantized fp8 weights instead.
DEVICE_CAST_W8 = True

# test-harness knobs (ignored in normal use)
TRACE = False
LAST_EXEC_NS = []
LAST_RESULTS = {}

_cache = {}


def _warmup_pe(nc, pool, ps_pool, n_mm, tag="ps"):
    """Dummy bf16 matmuls on scratch data, issued at kernel start so the
    PE's HAM clock-gate ramps toward 2.4 GHz while the input DMAs
    stream in."""
    wt = pool.tile([P, 512], BF16, name="warm_sb")
    nc.vector.memset(wt[:], 1.0)
    wp = ps_pool.tile([P, 512], F32, name="warm_ps", tag=tag)
    for _ in range(n_mm):
        nc.tensor.matmul(wp[:], wt[:, :P], wt[:], start=True, stop=True)
    return wt, wp


def _build_mixed(CB, C8, DIN, DOUT, EPC):
    """Per-core expert compute, mixed bf16/fp8-DoubleRow.

    Inputs : xbT [EPC, P, KT, CB]   bf16 (pre-scaled tokens, SBUF layout)
             x8T [EPC, P, KT, C8]   f8e4 (pre-scaled tokens, SBUF layout)
             wb  [EPC, P, KT, DOUT] bf16
             (w8 [EPC, P, KT, DOUT] f8e4 -- only if not DEVICE_CAST_W8)
    Output : yout [EPC, 2, MT, P, DOUT] bf16  (path 0 = bf16, 1 = fp8)
    """
    key = ("mix", CB, C8, DIN, DOUT, EPC, DEVICE_CAST_W8)
    if key in _cache:
        return _cache[key]
    KT = DIN // P
    MTB = CB // P
    MT8 = C8 // P
    NF = 512
    assert DOUT % NF == 0 and KT % 2 == 0
    NT = DOUT // NF
    assert EPC == 2
    nc = bacc.Bacc("TRN2", target_bir_lowering=False, debug=False,
                   num_devices=NCORES)
    xbT = nc.dram_tensor("xbT", [EPC, P, KT, CB], BF16, kind="ExternalInput")
    x8T = nc.dram_tensor("x8T", [EPC, P, KT, C8], F8, kind="ExternalInput")
    wb = nc.dram_tensor("wb", [EPC, P, KT, DOUT], BF16, kind="ExternalInput")
    if not DEVICE_CAST_W8:
        w8 = nc.dram_tensor("w8", [EPC, P, KT, DOUT], F8,
                            kind="ExternalInput")
    yout = nc.dram_tensor("yout", [EPC, 2, max(MTB, MT8), P, DOUT], BF16,
                          kind="ExternalOutput")

    with tile.TileContext(nc) as tc:
        with ExitStack() as ctx:
            in_pool = ctx.enter_context(tc.tile_pool(name="in", bufs=1))
            out_pool = ctx.enter_context(tc.tile_pool(name="out", bufs=12))
            ps = ctx.enter_context(tc.tile_pool(name="ps", bufs=8,
                                                space="PSUM"))
            warm_pool = ctx.enter_context(tc.tile_pool(name="warm", bufs=1))
            _warmup_pe(nc, warm_pool, ps, 16, tag="ps")

            xb_ts, x8_ts, wb_ts, w8_ts = [], [], [], []
            for e in range(EPC):
                xb_ts.append(in_pool.tile([P, KT, CB], BF16, name=f"xb{e}"))
                x8_ts.append(in_pool.tile([P, KT, C8], F8, name=f"x8{e}"))
                wb_ts.append(in_pool.tile([P, KT, DOUT], BF16,
                                          name=f"wb{e}"))
                w8_ts.append(in_pool.tile([P, KT, DOUT], F8, name=f"w8{e}"))

            # --- loads: compute-phase order, round-robin over queues ----
            # k-pair granularity so arrival tracks the k-outer waves.
            qs = [nc.sync, nc.scalar, nc.gpsimd]
            qi = [0]

            def load(dst, src):
                qs[qi[0] % 3].dma_start(dst, src)
                qi[0] += 1

            # phase 1: bf16(e0) — wb/xb k-pairs interleaved
            for k in range(0, KT, 2):
                load(wb_ts[0][:, k:k + 2], wb[0, :, k:k + 2])
                load(xb_ts[0][:, k:k + 2], xbT[0, :, k:k + 2])
            # phase 2: fp8(e0)
            load(x8_ts[0][:, 0:4], x8T[0, :, 0:4])
            load(x8_ts[0][:, 4:KT], x8T[0, :, 4:KT])
            if not DEVICE_CAST_W8:
                load(w8_ts[0][:, 0:4], w8[0, :, 0:4])
                load(w8_ts[0][:, 4:KT], w8[0, :, 4:KT])
            # phase 3: bf16(e1)
            for k in range(0, KT, 2):
                load(wb_ts[1][:, k:k + 2], wb[1, :, k:k + 2])
                load(xb_ts[1][:, k:k + 2], xbT[1, :, k:k + 2])
            # phase 4: fp8(e1)
            load(x8_ts[1][:, 0:4], x8T[1, :, 0:4])
            load(x8_ts[1][:, 4:KT], x8T[1, :, 4:KT])
            if not DEVICE_CAST_W8:
                load(w8_ts[1][:, 0:4], w8[1, :, 0:4])
                load(w8_ts[1][:, 4:KT], w8[1, :, 4:KT])

            def cast_w8(e, eng=None):
                # bf16 -> fp8 cast, k-pair granularity (|W| << 240, so a
                # plain cast cannot overflow e4m3). DVE tensor_copy or
                # ACT activation-copy, whichever engine has slack.
                for k in range(0, KT, 2):
                    if eng is nc.scalar:
                        eng.copy(w8_ts[e][:, k:k + 2],
                                 wb_ts[e][:, k:k + 2])
                    else:
                        nc.vector.tensor_copy(w8_ts[e][:, k:k + 2],
                                              wb_ts[e][:, k:k + 2])

            if DEVICE_CAST_W8:
                cast_w8(0)

            # --- compute phases ----------------------------------------
            store_ctr = [0]

            def emit_phase(e, path, waves, last_phase=False):
                f8p = (path == 1)
                x_t = (x8_ts if f8p else xb_ts)[e]
                w_t = (w8_ts if f8p else wb_ts)[e]
                MT = MT8 if f8p else MTB
                groups = [(m, n) for m in range(MT) for n in range(NT)]
                gi0 = 0
                out_tiles = {}
                nwaves = len(waves)
                for wi, wsize in enumerate(waves):
                    if isinstance(wsize, list):
                        wave = wsize
                        gi0 += len(wave)
                    else:
                        wave = groups[gi0:gi0 + wsize]
                        gi0 += wsize
                    pss = {g: ps.tile([P, NF], F32, tag="ps",
                                      name=f"ps_{e}_{path}_{g[0]}_{g[1]}")
                           for g in wave}
                    if f8p:
                        for kk in range(KT // 2):
                            for (m, n) in wave:
                                nc.tensor.matmul(
                                    pss[(m, n)][:],
                                    x_t[:, 2 * kk:2 * kk + 2,
                                        m * P:(m + 1) * P],
                                    w_t[:, 2 * kk:2 * kk + 2,
                                        n * NF:(n + 1) * NF],
                                    start=(kk == 0),
                                    stop=(kk == KT // 2 - 1),
                                    perf_mode=mybir.MatmulPerfMode.DoubleRow,
                                )
                    else:
                        for k in range(KT):
                            for (m, n) in wave:
                                nc.tensor.matmul(
                                    pss[(m, n)][:],
                                    x_t[:, k, m * P:(m + 1) * P],
                                    w_t[:, k, n * NF:(n + 1) * NF],
                                    start=(k == 0),
                                    stop=(k == KT - 1),
                                )
                    last_wave = last_phase and wi == nwaves - 1
                    fine = last_phase and wi >= nwaves - 2
                    for gi, (m, n) in enumerate(wave):
                        if m not in out_tiles:
                            out_tiles[m] = out_pool.tile(
                                [P, DOUT], BF16, tag="out",
                                name=f"out_{e}_{path}_{m}")
                        ot = out_tiles[m]
                        if last_wave and gi % 2 == 1:
                            nc.scalar.copy(ot[:, n * NF:(n + 1) * NF],
                                           pss[(m, n)][:])
                        else:
                            nc.vector.tensor_copy(
                                ot[:, n * NF:(n + 1) * NF], pss[(m, n)][:])
                        if fine:
                            # drain fine-grained: store each n-half right
                            # after its eviction, spread across the two
                            # HWDGE queues (the n0 halves are early and
                            # off the critical path; the final n1 halves
                            # drain in parallel).
                            eng = nc.sync if gi % 2 == 0 else nc.scalar
                            eng.dma_start(yout[e, path, m, :,
                                               n * NF:(n + 1) * NF],
                                          ot[:, n * NF:(n + 1) * NF])
                        elif n == NT - 1:
                            eng = qs[store_ctr[0] % 3]
                            store_ctr[0] += 1
                            eng.dma_start(yout[e, path, m], ot[:])

            # bf16(e0) first wave of 6: chunk consumption (1.3us/chunk)
            # then matches the early DMA arrival rate, avoiding PE stalls
            # while the rings ramp up.
            emit_phase(0, 0, [7, 1])
            emit_phase(0, 1, [4, 4])
            if DEVICE_CAST_W8:
                cast_w8(1, eng=nc.scalar)
            emit_phase(1, 0, [4, 4])
            emit_phase(1, 1, [4, [(2, 0), (3, 0)], [(2, 1), (3, 1)]],
                       last_phase=True)
    nc.compile()
    _cache[key] = nc
    return nc


def _run(nc, in_maps):
    kw = {}
    if TRACE:
        kw["trace"] = True
    res = run_bass_kernel_spmd(nc, in_maps, list(range(NCORES)), **kw)
    if TRACE:
        LAST_EXEC_NS.append(res.exec_time_ns)
        LAST_RESULTS["last"] = res
    return res.results


def _pack(a2d, KT, C, np_dtype):
    """[DIN, n] f32 -> [P, KT, C] np_dtype, zero-padded along tokens."""
    out = np.zeros((P, KT, C), np_dtype)
    n = a2d.shape[1]
    out[:, :, :n] = (a2d.reshape(KT, P, -1).transpose(1, 0, 2)
                     .astype(np_dtype))
    return out


def kernel(x, gate_w, gate_b, expert_w, expert_b, topk):
    x = np.ascontiguousarray(np.asarray(x, dtype=np.float32))
    gate_w = np.asarray(gate_w, dtype=np.float32)
    gate_b = np.asarray(gate_b, dtype=np.float32)
    expert_w = np.asarray(expert_w, dtype=np.float32)
    expert_b = np.asarray(expert_b, dtype=np.float32)
    topk = int(topk)

    B, DIN = x.shape
    E, _, DOUT = expert_w.shape
    assert B % P == 0 and DIN % P == 0
    EPC = E // NCORES
    assert EPC * NCORES == E
    KT = DIN // P

    # ---- host: gating (softmax + top-k) in float64 ----
    logits = x.astype(np.float64) @ gate_w.astype(np.float64).T \
        + gate_b.astype(np.float64)
    order = np.argsort(-logits, axis=1, kind="stable")[:, :topk]
    z = np.exp(logits - logits.max(axis=1, keepdims=True))
    probs = z / z.sum(axis=1, keepdims=True)
    pv = np.take_along_axis(probs, order, axis=1).astype(np.float32)

    # capacity: split the mean per-expert load between the two paths
    cap = (max(P, B * topk // E) // P) * P
    CB = C8 = cap // 2

    # ---- host: routing; per expert sort by p, split bf16/fp8/host ----
    dev_b, dev_8, host_t = [], [], []
    for e in range(E):
        selmask = (order == e)
        t = np.nonzero(selmask.any(axis=1))[0]
        p = np.where(selmask[t, 0], pv[t, 0],
                     pv[t, 1] if topk > 1 else 0.0)
        o = np.argsort(-p, kind="stable")
        t, p = t[o], p[o]
        dev_b.append((t[:CB], p[:CB]))
        dev_8.append((t[CB:CB + C8], p[CB:CB + C8]))
        host_t.append((t[CB + C8:], p[CB + C8:]))

    nc = _build_mixed(CB, C8, DIN, DOUT, EPC)
    in_maps = []
    for c in range(NCORES):
        xbT = np.zeros((EPC, P, KT, CB), NPBF16)
        x8T = np.zeros((EPC, P, KT, C8), NPF8)
        wbp = np.zeros((EPC, P, KT, DOUT), NPBF16)
        w8p = None if DEVICE_CAST_W8 else \
            np.zeros((EPC, P, KT, DOUT), NPF8)
        for j in range(EPC):
            e = EPC * c + j
            tb, pb = dev_b[e]
            t8, p8 = dev_8[e]
            if len(tb):
                xbT[j] = _pack((x[tb] * pb[:, None]).T, KT, CB, NPBF16)
            if len(t8):
                x8T[j] = _pack((x[t8] * p8[:, None]).T, KT, C8, NPF8)
            wf = expert_w[e].reshape(KT, P, DOUT).transpose(1, 0, 2)
            wbp[j] = wf.astype(NPBF16)
            if w8p is not None:
                w8p[j] = wf.astype(NPF8)
        im = {"xbT": xbT, "x8T": x8T, "wb": wbp}
        if w8p is not None:
            im["w8"] = w8p
        in_maps.append(im)
    r = _run(nc, in_maps)

    # ---- host: scatter-add outputs (pure adds; both paths pre-scaled) --
    y = np.zeros((B, DOUT), np.float32)
    for c in range(NCORES):
        yo = np.asarray(r[c]["yout"])
        for j in range(EPC):
            e = EPC * c + j
            tb, _ = dev_b[e]
            t8, _ = dev_8[e]
            if len(tb):
                y[tb] += yo[j, 0].reshape(-1, DOUT)[:len(tb)] \
                    .astype(np.float32)
            if len(t8):
                y[t8] += yo[j, 1].reshape(-1, DOUT)[:len(t8)] \
                    .astype(np.float32)
    for e in range(E):
        t, p = host_t[e]
        if len(t):
            y[t] += (x[t] * p[:, None]) @ expert_w[e]
    if np.any(expert_b):
        for e in range(E):
            for (t, p) in (dev_b[e], dev_8[e], host_t[e]):
                if len(t):
                    y[t] += p[:, None] * expert_b[e][None, :]
    return y


# revision 20
# speedup vs baseline: 1.3108x; 1.0252x over previous
"""MoE routing kernel for Trainium2, 8 NeuronCores.

Strategy (expert-parallel, mixed precision, one device launch):
  Host: gating softmax + top-k in float64 (selection is exact vs the
  f32 reference since top-k margins dwarf f32 rounding noise). Per
  expert, sort its assigned tokens by gate value p (descending):
    - top CB=512 (large p)  -> bf16 path (gate-pre-scaled tokens)
    - next C8=512 (small p) -> fp8 e4m3 path (gate-pre-scaled), run
      with MatmulPerfMode.DoubleRow: 2x PE throughput. The fp8
      quantization error lands only on the low-gate half of the
      assignments, keeping total L2 error ~1.6e-2 (< 2e-2 gate).
    - remainder (~2%)       -> host f32 (standard capacity overflow)
  All tensors are packed on the host directly into SBUF layout
  [P, KT, free] so DMA descriptors have multi-KB contiguous runs per
  partition. All DMA queues share ~320 GB/s aggregate, so loads are
  emitted in compute-phase order, round-robin across the 3 queues
  (sync/scalar HWDGE + gpsimd SWDGE). The fp8 copies of the expert
  weights are produced ON DEVICE (DVE bf16->fp8 cast) instead of
  being loaded, cutting input traffic 9MB -> 7MB per core.
  Device phase order bf16(e0), fp8(e0), bf16(e1), fp8(e1): the long
  bf16 phases lead, giving the DMA stream runway; fp8 phases then
  only need the small x8 tensors plus the already-resident weights.
  fp32 PSUM; bf16 outputs (halves store traffic); warmup matmuls
  cover the PE clock ramp; small final waves keep the drain short.

PE floor: per core 128 bf16 + 64 DR matmul instrs at ~216ns issue
rate ~ 41.5us. Measured baseline (all-bf16) was ~76-81us.
"""
import numpy as np
from contextlib import ExitStack

import ml_dtypes

import concourse.mybir as mybir
from concourse import bacc, tile
from concourse.bass_utils import run_bass_kernel_spmd

NCORES = 8
P = 128
F32 = mybir.dt.float32
BF16 = mybir.dt.bfloat16
F8 = mybir.dt.float8e4
NPBF16 = ml_dtypes.bfloat16
NPF8 = ml_dtypes.float8_e4m3

# device-side bf16->fp8 weight cast (saves 2MB/core of input DMA).
# Set False to load host-quantized fp8 weights instead.
DEVICE_CAST_W8 = True

# test-harness knobs (ignored in normal use)
TRACE = False
LAST_EXEC_NS = []
LAST_RESULTS = {}

_cache = {}


def _warmup_pe(nc, pool, ps_pool, n_mm, tag="ps"):
    """Dummy bf16 matmuls on scratch data, issued at kernel start so the
    PE's HAM clock-gate ramps toward 2.4 GHz while the input DMAs
    stream in."""
    wt = pool.tile([P, 512], BF16, name="warm_sb")
    nc.vector.memset(wt[:], 1.0)
    wp = ps_pool.tile([P, 512], F32, name="warm_ps", tag=tag)
    for _ in range(n_mm):
        nc.tensor.matmul(wp[:], wt[:, :P], wt[:], start=True, stop=True)
    return wt, wp


def _build_mixed(CB, C8, DIN, DOUT, EPC):
    """Per-core expert compute, mixed bf16/fp8-DoubleRow.

    Inputs : xbT [EPC, P, KT, CB]   bf16 (pre-scaled tokens, SBUF layout)
             x8T [EPC, P, KT, C8]   f8e4 (pre-scaled tokens, SBUF layout)
             wb  [EPC, P, KT, DOUT] bf16
             (w8 [EPC, P, KT, DOUT] f8e4 -- only if not DEVICE_CAST_W8)
    Output : yout [EPC, 2, MT, P, DOUT] bf16  (path 0 = bf16, 1 = fp8)
    """
    key = ("mix", CB, C8, DIN, DOUT, EPC, DEVICE_CAST_W8)
    if key in _cache:
        return _cache[key]
    KT = DIN // P
    MTB = CB // P
    MT8 = C8 // P
    NF = 512
    assert DOUT % NF == 0 and KT % 2 == 0
    NT = DOUT // NF
    assert EPC == 2
    nc = bacc.Bacc("TRN2", target_bir_lowering=False, debug=False,
                   num_devices=NCORES)
    xbT = nc.dram_tensor("xbT", [EPC, P, KT, CB], BF16, kind="ExternalInput")
    x8T = nc.dram_tensor("x8T", [EPC, P, KT, C8], F8, kind="ExternalInput")
    wb = nc.dram_tensor("wb", [EPC, P, KT, DOUT], BF16, kind="ExternalInput")
    if not DEVICE_CAST_W8:
        w8 = nc.dram_tensor("w8", [EPC, P, KT, DOUT], F8,
                            kind="ExternalInput")
    yout = nc.dram_tensor("yout", [EPC, 2, max(MTB, MT8), P, DOUT], BF16,
                          kind="ExternalOutput")

    with tile.TileContext(nc) as tc:
        with ExitStack() as ctx:
            in_pool = ctx.enter_context(tc.tile_pool(name="in", bufs=1))
            out_pool = ctx.enter_context(tc.tile_pool(name="out", bufs=12))
            ps = ctx.enter_context(tc.tile_pool(name="ps", bufs=8,
                                                space="PSUM"))
            warm_pool = ctx.enter_context(tc.tile_pool(name="warm", bufs=1))
            _warmup_pe(nc, warm_pool, ps, 16, tag="ps")

            xb_ts, x8_ts, wb_ts, w8_ts = [], [], [], []
            for e in range(EPC):
                xb_ts.append(in_pool.tile([P, KT, CB], BF16, name=f"xb{e}"))
                x8_ts.append(in_pool.tile([P, KT, C8], F8, name=f"x8{e}"))
                wb_ts.append(in_pool.tile([P, KT, DOUT], BF16,
                                          name=f"wb{e}"))
                w8_ts.append(in_pool.tile([P, KT, DOUT], F8, name=f"w8{e}"))

            # --- loads: compute-phase order, round-robin over queues ----
            # k-pair granularity so arrival tracks the k-outer waves.
            qs = [nc.sync, nc.scalar, nc.gpsimd]
            qi = [0]

            def load(dst, src):
                qs[qi[0] % 3].dma_start(dst, src)
                qi[0] += 1

            # phase 1: bf16(e0) — strict k-order, equal bytes per queue
            # per k-pair (wb single-k 256KB + xb pair 256KB), so arrival
            # order across the shared-bandwidth queues matches the
            # k-outer consumption order of the first wave.
            for k in range(0, KT, 2):
                load(wb_ts[0][:, k:k + 1], wb[0, :, k:k + 1])
                load(wb_ts[0][:, k + 1:k + 2], wb[0, :, k + 1:k + 2])
                load(xb_ts[0][:, k:k + 2], xbT[0, :, k:k + 2])
            # phase 2: fp8(e0)
            load(x8_ts[0][:, 0:4], x8T[0, :, 0:4])
            load(x8_ts[0][:, 4:KT], x8T[0, :, 4:KT])
            if not DEVICE_CAST_W8:
                load(w8_ts[0][:, 0:4], w8[0, :, 0:4])
                load(w8_ts[0][:, 4:KT], w8[0, :, 4:KT])
            # phase 3: bf16(e1)
            for k in range(0, KT, 2):
                load(wb_ts[1][:, k:k + 2], wb[1, :, k:k + 2])
                load(xb_ts[1][:, k:k + 2], xbT[1, :, k:k + 2])
            # phase 4: fp8(e1)
            load(x8_ts[1][:, 0:4], x8T[1, :, 0:4])
            load(x8_ts[1][:, 4:KT], x8T[1, :, 4:KT])
            if not DEVICE_CAST_W8:
                load(w8_ts[1][:, 0:4], w8[1, :, 0:4])
                load(w8_ts[1][:, 4:KT], w8[1, :, 4:KT])

            def cast_w8(e, eng=None):
                # bf16 -> fp8 cast, k-pair granularity (|W| << 240, so a
                # plain cast cannot overflow e4m3). DVE tensor_copy or
                # ACT activation-copy, whichever engine has slack.
                for k in range(0, KT, 2):
                    if eng is nc.scalar:
                        eng.copy(w8_ts[e][:, k:k + 2],
                                 wb_ts[e][:, k:k + 2])
                    else:
                        nc.vector.tensor_copy(w8_ts[e][:, k:k + 2],
                                              wb_ts[e][:, k:k + 2])

            if DEVICE_CAST_W8:
                cast_w8(0)

            # --- compute phases ----------------------------------------
            store_ctr = [0]

            def emit_phase(e, path, waves, last_phase=False):
                f8p = (path == 1)
                x_t = (x8_ts if f8p else xb_ts)[e]
                w_t = (w8_ts if f8p else wb_ts)[e]
                MT = MT8 if f8p else MTB
                groups = [(m, n) for m in range(MT) for n in range(NT)]
                gi0 = 0
                out_tiles = {}
                nwaves = len(waves)
                for wi, wsize in enumerate(waves):
                    if isinstance(wsize, list):
                        wave = wsize
                        gi0 += len(wave)
                    else:
                        wave = groups[gi0:gi0 + wsize]
                        gi0 += wsize
                    pss = {g: ps.tile([P, NF], F32, tag="ps",
                                      name=f"ps_{e}_{path}_{g[0]}_{g[1]}")
                           for g in wave}
                    if f8p:
                        for kk in range(KT // 2):
                            for (m, n) in wave:
                                nc.tensor.matmul(
                                    pss[(m, n)][:],
                                    x_t[:, 2 * kk:2 * kk + 2,
                                        m * P:(m + 1) * P],
                                    w_t[:, 2 * kk:2 * kk + 2,
                                        n * NF:(n + 1) * NF],
                                    start=(kk == 0),
                                    stop=(kk == KT // 2 - 1),
                                    perf_mode=mybir.MatmulPerfMode.DoubleRow,
                                )
                    else:
                        for k in range(KT):
                            for (m, n) in wave:
                                nc.tensor.matmul(
                                    pss[(m, n)][:],
                                    x_t[:, k, m * P:(m + 1) * P],
                                    w_t[:, k, n * NF:(n + 1) * NF],
                                    start=(k == 0),
                                    stop=(k == KT - 1),
                                )
                    last_wave = last_phase and wi == nwaves - 1
                    fine = last_phase and wi >= nwaves - 2
                    for gi, (m, n) in enumerate(wave):
                        if m not in out_tiles:
                            out_tiles[m] = out_pool.tile(
                                [P, DOUT], BF16, tag="out",
                                name=f"out_{e}_{path}_{m}")
                        ot = out_tiles[m]
                        if last_wave and gi % 2 == 1:
                            nc.scalar.copy(ot[:, n * NF:(n + 1) * NF],
                                           pss[(m, n)][:])
                        else:
                            nc.vector.tensor_copy(
                                ot[:, n * NF:(n + 1) * NF], pss[(m, n)][:])
                        if fine:
                            # drain fine-grained: store each n-half right
                            # after its eviction, spread across the two
                            # HWDGE queues (the n0 halves are early and
                            # off the critical path; the final n1 halves
                            # drain in parallel).
                            eng = nc.sync if gi % 2 == 0 else nc.scalar
                            eng.dma_start(yout[e, path, m, :,
                                               n * NF:(n + 1) * NF],
                                          ot[:, n * NF:(n + 1) * NF])
                        elif n == NT - 1:
                            eng = qs[store_ctr[0] % 3]
                            store_ctr[0] += 1
                            eng.dma_start(yout[e, path, m], ot[:])

            # bf16(e0) first wave of 6: chunk consumption (1.3us/chunk)
            # then matches the early DMA arrival rate, avoiding PE stalls
            # while the rings ramp up.
            emit_phase(0, 0, [7, 1])
            emit_phase(0, 1, [4, 4])
            if DEVICE_CAST_W8:
                cast_w8(1, eng=nc.scalar)
            emit_phase(1, 0, [4, 4])
            emit_phase(1, 1, [4, [(2, 0), (3, 0)], [(2, 1), (3, 1)]],
                       last_phase=True)
    nc.compile()
    _cache[key] = nc
    return nc


def _run(nc, in_maps):
    kw = {}
    if TRACE:
        kw["trace"] = True
    res = run_bass_kernel_spmd(nc, in_maps, list(range(NCORES)), **kw)
    if TRACE:
        LAST_EXEC_NS.append(res.exec_time_ns)
        LAST_RESULTS["last"] = res
    return res.results


def _pack(a2d, KT, C, np_dtype):
    """[DIN, n] f32 -> [P, KT, C] np_dtype, zero-padded along tokens."""
    out = np.zeros((P, KT, C), np_dtype)
    n = a2d.shape[1]
    out[:, :, :n] = (a2d.reshape(KT, P, -1).transpose(1, 0, 2)
                     .astype(np_dtype))
    return out


def kernel(x, gate_w, gate_b, expert_w, expert_b, topk):
    x = np.ascontiguousarray(np.asarray(x, dtype=np.float32))
    gate_w = np.asarray(gate_w, dtype=np.float32)
    gate_b = np.asarray(gate_b, dtype=np.float32)
    expert_w = np.asarray(expert_w, dtype=np.float32)
    expert_b = np.asarray(expert_b, dtype=np.float32)
    topk = int(topk)

    B, DIN = x.shape
    E, _, DOUT = expert_w.shape
    assert B % P == 0 and DIN % P == 0
    EPC = E // NCORES
    assert EPC * NCORES == E
    KT = DIN // P

    # ---- host: gating (softmax + top-k) in float64 ----
    logits = x.astype(np.float64) @ gate_w.astype(np.float64).T \
        + gate_b.astype(np.float64)
    order = np.argsort(-logits, axis=1, kind="stable")[:, :topk]
    z = np.exp(logits - logits.max(axis=1, keepdims=True))
    probs = z / z.sum(axis=1, keepdims=True)
    pv = np.take_along_axis(probs, order, axis=1).astype(np.float32)

    # capacity: split the mean per-expert load between the two paths
    cap = (max(P, B * topk // E) // P) * P
    CB = C8 = cap // 2

    # ---- host: routing; per expert sort by p, split bf16/fp8/host ----
    dev_b, dev_8, host_t = [], [], []
    for e in range(E):
        selmask = (order == e)
        t = np.nonzero(selmask.any(axis=1))[0]
        p = np.where(selmask[t, 0], pv[t, 0],
                     pv[t, 1] if topk > 1 else 0.0)
        o = np.argsort(-p, kind="stable")
        t, p = t[o], p[o]
        dev_b.append((t[:CB], p[:CB]))
        dev_8.append((t[CB:CB + C8], p[CB:CB + C8]))
        host_t.append((t[CB + C8:], p[CB + C8:]))

    nc = _build_mixed(CB, C8, DIN, DOUT, EPC)
    in_maps = []
    for c in range(NCORES):
        xbT = np.zeros((EPC, P, KT, CB), NPBF16)
        x8T = np.zeros((EPC, P, KT, C8), NPF8)
        wbp = np.zeros((EPC, P, KT, DOUT), NPBF16)
        w8p = None if DEVICE_CAST_W8 else \
            np.zeros((EPC, P, KT, DOUT), NPF8)
        for j in range(EPC):
            e = EPC * c + j
            tb, pb = dev_b[e]
            t8, p8 = dev_8[e]
            if len(tb):
                xbT[j] = _pack((x[tb] * pb[:, None]).T, KT, CB, NPBF16)
            if len(t8):
                x8T[j] = _pack((x[t8] * p8[:, None]).T, KT, C8, NPF8)
            wf = expert_w[e].reshape(KT, P, DOUT).transpose(1, 0, 2)
            wbp[j] = wf.astype(NPBF16)
            if w8p is not None:
                w8p[j] = wf.astype(NPF8)
        im = {"xbT": xbT, "x8T": x8T, "wb": wbp}
        if w8p is not None:
            im["w8"] = w8p
        in_maps.append(im)
    r = _run(nc, in_maps)

    # ---- host: scatter-add outputs (pure adds; both paths pre-scaled) --
    y = np.zeros((B, DOUT), np.float32)
    for c in range(NCORES):
        yo = np.asarray(r[c]["yout"])
        for j in range(EPC):
            e = EPC * c + j
            tb, _ = dev_b[e]
            t8, _ = dev_8[e]
            if len(tb):
                y[tb] += yo[j, 0].reshape(-1, DOUT)[:len(tb)] \
                    .astype(np.float32)
            if len(t8):
                y[t8] += yo[j, 1].reshape(-1, DOUT)[:len(t8)] \
                    .astype(np.float32)
    for e in range(E):
        t, p = host_t[e]
        if len(t):
            y[t] += (x[t] * p[:, None]) @ expert_w[e]
    if np.any(expert_b):
        for e in range(E):
            for (t, p) in (dev_b[e], dev_8[e], host_t[e]):
                if len(t):
                    y[t] += p[:, None] * expert_b[e][None, :]
    return y
